# revision 28
# baseline (speedup 1.0000x reference)
"""BPCA pooling layer on 8 Trainium2 NeuronCores (Bass/Tile).

Math: per sample, the reference's `data = patches.reshape(-1, 4)` groups 4
consecutive channels (C=256 is divisible by 4), so `data` is exactly the
sample's contiguous buffer viewed as [N, 4] with N = H*W*C/4.  The layer is:

  1. per-column mean/std over N rows, dn = (data-mean)/std
  2. gram = dn^T dn (4x4), comp = top eigenvector (jnp.linalg.eigh)
  3. out = (dn @ comp) reshaped to [H/2, W/2, C] with channel permutation
     c' = (2*di+dj)*64 + (c//4)

Device plan (2 samples per core, pure data parallel).  Both passes are
DMA-bound, so all device I/O except the tiny stats tensor is fp16 --
quantizing x to fp16 perturbs the final output by ~3e-4 rel (measured
against the fixed seed), 60x under the 2e-2 gate, and halves traffic:

  pass 1: PE computes the 128x128 half-channel second-moment matrix
          M[j,j'] = sum_{pix,G} x[pix,128G+j]*x[pix,128G+j'] plus column
          sums (ones column), accumulated in fp32 PSUM, from fp16 inputs.
          128-col blocks (one matmul per 128-row block, N=130 moving)
          instead of 256-col halves: same LDWEIGHTS count but half the
          moving columns, so the PE stream (~81ns/MM warm) stays under
          the DMA stream.
  host:   fold M into the 4x4 gram (S_kl = sum_g M[4g+k,4g+l]), compute
          mean/std/gram in f64, eigh on CPU jax (same implementation the
          reference uses), derive w_k = comp_k/std_k and
          bias = -sum_k mean_k*comp_k/std_k.
  pass 2: out = sum of four host-prescaled k-planes (the host builds the
          k-plane layout after pass 1, when it already has w/bias, so the
          per-plane scale w_k and the bias ride that existing layout
          pass) -- three tensor_tensor adds on DVE (2x mode for packed
          fp16), far under the DMA stream.  NOT a scalar_tensor_tensor
          chain: STT has no accelerated DVE uops and runs 1 elem/cyc.
          Output channel permutation is folded into the host layout.

All bulk loads go through the single SP DMA queue: one queue aggregates
~350 GB/s across the 16 DMA engines, while splitting across two queues
measured LOWER total (the engines, not the queue, are the resource);
the ACT queue only carries small stores (it gets poor engine service).
Graduated tile ladders (small head, small tail) plus bufs=8 prefetch
keep the whole-tile DMA-completion semaphores off the critical path.
"""

import numpy as np

# ---------------------------------------------------------------------------
# Problem constants (hardcoded per spec)
# ---------------------------------------------------------------------------
B, H, W, C = 16, 112, 112, 256
N_CORES = 8
SPC = B // N_CORES          # samples per core = 2
PIX = H * W                 # 12544 pixels per sample
NBLK1 = PIX * C // (128 * 128)  # 196 row-blocks of 128 per sample
BST1 = 130                  # per-block SBUF cols: 128 data + 1 ones + 1 pad
P1_TILES = [32, 32, 32, 32, 32, 24, 12]  # (sum=196)
# uniform 32-block tiles keep 8.3KB per-partition DMA segments through the
# stream (the old graduated head ran 0.5-5KB segments and measurably
# dropped the stream rate); the shrinking tail bounds the PE chase after
# the last tile lands
NROWS = PIX * C // 4        # 802816 rows of the [N, 4] data matrix
HO, WO = H // 2, W // 2     # 56 x 56 output

_programs = None
_fused_program = None
LAST_PROFILE = {}
TRACE = False
TRACE_DIRS = {}
FUSED = True                # single NEFF: pays the ~9us queue-startup head
                            # and the ~2.3us end fence once instead of twice
NSQ = 9                     # matrix squarings (power 512; sim err 8e-5)
P2_TILES = [2, 4, 8, 12, 12, 9, 2]
FP2_TILES = [7] * 7         # fused plane tiles: uniform 14.3KB segments
CONST_COLS = 332
P1_CONST_COLS = 146


# ---------------------------------------------------------------------------
# TileContext with a walrus-compatible tail drain
# ---------------------------------------------------------------------------
def _make_tile_context(nc):
    from concourse.tile import TileContext
    return TileContext(nc)


def _split_sync_waits(nc):
    """walrus (CoreV2/V3 codegen) rejects instructions carrying more than 2
    sync commands (waits + updates combined); Tile freely emits e.g. 2 waits
    + 1 update.  Hoist excess waits onto same-engine NOPs inserted directly
    before the offending instruction -- same engine means the same program-
    order point, so semantics are unchanged."""
    import concourse.mybir as mybir

    def mint_nop(engine):
        inner = nc.engines[engine].nop().ins
        for blk in nc.m.functions[0].blocks:
            il = blk.instructions
            for k in range(len(il) - 1, -1, -1):
                if il[k] is inner:
                    il.pop(k)
                    return inner
        raise RuntimeError("minted nop not found in any block")

    for fn in nc.m.functions:
        for blk in fn.blocks:
            il = blk.instructions
            i = 0
            while i < len(il):
                inst = il[i]
                si = inst.sync_info
                waits = list(si.on_wait) if si and si.on_wait else []
                upds = list(si.on_update) if si and si.on_update else []
                # observed walrus limits: at most 1 wait per instruction
                # (1 wait + 1 update compiles; 2 waits anywhere does not)
                if len(waits) > 1:
                    extra, keep = waits[:-1], waits[-1:]
                    for wchunk in extra:
                        nop = mint_nop(inst.engine)
                        nop.sync_info = mybir.SyncInfo(
                            on_wait=[wchunk], on_update=[])
                        il.insert(i, nop)
                        i += 1
                    inst.sync_info = mybir.SyncInfo(
                        on_wait=keep, on_update=upds)
                i += 1


def _p1_consts():
    """[128, 146] f32: cols 0:130 block-diag fold mask (col 128 = ones for
    the chansum column, col 129 = 0 to kill the pad col), 130:138 sample-0
    row selector (p%4==m in col 130+m), 138:146 sample-1 selector (col
    142+m)."""
    ct = np.zeros((128, P1_CONST_COLS), np.float32)
    p = np.arange(128)
    q = np.arange(128)
    ct[:, 0:128] = (p[:, None] // 4 == q[None, :] // 4).astype(np.float32)
    ct[:, 128] = 1.0
    for m in range(4):
        ct[:, 130 + m] = (p % 4 == m)
        ct[:, 142 + m] = (p % 4 == m)
    return ct


def _build_pass1():
    import concourse.bass as bass
    import concourse.mybir as mybir

    f32 = mybir.dt.float32
    f16 = mybir.dt.float16
    alu = mybir.AluOpType

    nc = bass.Bass("TRN2", target_bir_lowering=False, debug=False,
                   num_devices=N_CORES)
    # The host pre-interleaves a ones column per block (col 128 of each
    # 130-wide block) so one DMA loads data + ones and no on-device memset
    # is needed.
    x = nc.dram_tensor("x", [SPC, 128, NBLK1 * BST1], f16,
                       kind="ExternalInput").ap()
    cst = nc.dram_tensor("cst", [128, P1_CONST_COLS], f32,
                         kind="ExternalInput").ap()
    # folded stats: rows 4s+m hold M_s[4*(col//4)+m, col] for col<128 (an
    # exact selector-matmul copy of the block-diag entries the host fold
    # uses) and colsum_s[m] in col 128.  4KB store instead of the old
    # 2x[128,130] (255KB) whose 288-packet drain on the scalar queue was
    # ~5us of critical-path tail.
    stats = nc.dram_tensor("stats", [8, BST1], f32,
                           kind="ExternalOutput").ap()

    with _make_tile_context(nc) as tc:
        with (
            tc.tile_pool(name="cstp", bufs=1) as cstp,
            tc.tile_pool(name="inp", bufs=8) as inp,
            tc.tile_pool(name="psum", bufs=2, space="PSUM") as psum,
            tc.tile_pool(name="pst", bufs=1, space="PSUM") as pst,
            tc.tile_pool(name="sml", bufs=1) as sml,
        ):
            # const load on the ACT queue: the load queue stays clean and
            # the transfer (75KB) completes long before the first fold use
            ct = cstp.tile([128, P1_CONST_COLS], f32, tag="cst")
            nc.scalar.dma_start(out=ct[:], in_=cst[:])
            mask130 = ct[:, 0:130]
            lsels = [ct[:, 130:138], ct[:, 138:146]]

            bms = []
            for s in range(SPC):
                ps = psum.tile([128, BST1], f32, tag="ps")
                b0 = 0
                for nb in P1_TILES:
                    t = inp.tile([128, nb * BST1], f16, tag="in")
                    t3 = t[:].rearrange("p (j b) -> p j b", b=BST1)
                    nc.sync.dma_start(
                        out=t[:],
                        in_=x[s, :, b0 * BST1:(b0 + nb) * BST1])
                    for j in range(nb):
                        first = b0 + j == 0
                        last = b0 + j == NBLK1 - 1
                        nc.tensor.matmul(ps[:, 0:BST1],
                                         t3[:, j:j + 1, 0:128],
                                         t3[:, j:j + 1, 0:BST1],
                                         start=first, stop=last,
                                         skip_group_check=True)
                    b0 += nb
                # block-diag mask applied straight from PSUM (sample 0's
                # runs mid-stream on the idle DVE)
                bm = sml.tile([128, BST1], f32, tag=f"bm{s}")
                nc.vector.tensor_tensor(bm[:], ps[:, 0:BST1], mask130,
                                        op=alu.mult)
                bms.append(bm)
            # both selector matmuls at the end so the PE gram streams are
            # never interrupted by a wait on the DVE mask-mult
            t1p = pst.tile([8, BST1], f32, tag="t1p")
            for s in range(SPC):
                nc.tensor.matmul(t1p[:], lsels[s], bms[s][:],
                                 start=(s == 0), stop=(s == SPC - 1),
                                 skip_group_check=True)
            t1b = sml.tile([8, BST1], f32, tag="t1b")
            nc.vector.tensor_copy(out=t1b[:], in_=t1p[:])
            # the load queue is idle once the last load retires, so this
            # drains immediately (the ACT queue adds ~3us of service lag)
            nc.sync.dma_start(out=stats[:], in_=t1b[:])
    _split_sync_waits(nc)
    return nc


def _build_pass2():
    import concourse.bass as bass
    import concourse.mybir as mybir

    f16 = mybir.dt.float16
    alu = mybir.AluOpType
    OO = 49  # output pixels per partition (3136 = 64 partitions x 49)

    nc = bass.Bass("TRN2", target_bir_lowering=False, debug=False,
                   num_devices=N_CORES)
    # Partition p = (s_local*64 + p64): both samples fill 128 partitions so
    # every DMA is a fully-contiguous 128-partition transfer.  The host
    # builds the k-plane layout AFTER pass 1 (it already has w/bias then),
    # so the per-plane scale w_k and the bias (folded into plane 0) ride
    # the existing host layout pass; the device sums the four planes with
    # three tensor_tensor adds (2x DVE mode for packed fp16).
    x = nc.dram_tensor("x", [128, OO * 4 * C], f16,
                       kind="ExternalInput").ap()
    out = nc.dram_tensor("out", [128, OO * C], f16,
                         kind="ExternalOutput").ap()

    with _make_tile_context(nc) as tc:
        with (
            tc.tile_pool(name="inp", bufs=4) as inp,
            tc.tile_pool(name="acc", bufs=2) as accp,
            tc.tile_pool(name="otp", bufs=4) as otp,
        ):
            off = 0
            ooff = 0
            for ti, oo in enumerate(P2_TILES):
                F = oo * C
                it = inp.tile([128, 4 * F], f16, tag="it")
                nc.sync.dma_start(out=it[:], in_=x[:, off:off + 4 * F])
                u0 = accp.tile([128, F], f16, tag="u0")
                u1 = accp.tile([128, F], f16, tag="u1")
                # deep ot pool: DVE never waits on ACT-queue store service
                ot = otp.tile([128, F], f16, tag="ot")
                nc.vector.tensor_tensor(
                    u0[:], it[:, 0:F], it[:, F:2 * F], op=alu.add)
                nc.vector.tensor_tensor(
                    u1[:], it[:, 2 * F:3 * F], it[:, 3 * F:4 * F],
                    op=alu.add)
                nc.vector.tensor_tensor(
                    ot[:], u0[:], u1[:], op=alu.add)
                if ti < len(P2_TILES) - 2:
                    # mid-stream stores on the ACT queue: the load queue
                    # stays free to prefetch
                    nc.scalar.dma_start(
                        out=out[:, ooff:ooff + F], in_=ot[:])
                else:
                    # the last stores ride the load queue, which is idle
                    # once the final load descriptor retires and drains
                    # far faster than the ACT queue
                    nc.sync.dma_start(
                        out=out[:, ooff:ooff + F], in_=ot[:])
                off += 4 * F
                ooff += F
    _split_sync_waits(nc)
    return nc


R_PROBE = (0.5393, -0.2117, 0.8313, 0.1078)  # fixed eig probe (per k)
ESHIFT = 0.02                                # a8 = gram/(ESHIFT*N) - 49*I


def _host_consts():
    """Constant tensor for the fused kernel's on-device fold/eigensolve."""
    ct = np.zeros((128, CONST_COLS), np.float32)
    p = np.arange(128)
    q = np.arange(128)
    p8 = np.arange(8)
    # 0:130 block-diag mask for M -> per-group fold (col 128 = chansums)
    ct[:, 0:128] = (p[:, None] // 4 == q[None, :] // 4).astype(np.float32)
    ct[:, 128] = 1.0
    # 130:138 / 138:146 per-sample fold selectors (PSUM-accumulated);
    # rows 0:8 of 130:134 double as lseld (p%4==l diag selector)
    for m in range(4):
        ct[:, 130 + m] = (p % 4 == m)
        ct[:, 142 + m] = (p % 4 == m)
    # 146:154 I8, 154:162 blockones8
    ct[0:8, 146:154] = np.eye(8, dtype=np.float32)
    ct[0:8, 154:162] = (p8[:, None] // 4 == p8[None, :] // 4)
    # 162 r8 (fixed probe vector, repeated per sample)
    ct[0:8, 162] = np.array(R_PROBE, np.float32)[p8 % 4]
    # 163:171 / 171:179 half-diagonal projectors (block-diag embed of S)
    ct[0:8, 163:171] = np.diag((p8 < 4).astype(np.float32))
    ct[0:8, 171:179] = np.diag((p8 >= 4).astype(np.float32))
    # 179:187 -49*I8; 187:195 m8/(ESHIFT*N); 195:203 -m8/ESHIFT
    ct[0:8, 179:187] = -49.0 * np.eye(8, dtype=np.float32)
    ct[0:8, 187:195] = ct[0:8, 154:162] / (ESHIFT * NROWS)
    ct[0:8, 195:203] = ct[0:8, 154:162] * (-1.0 / ESHIFT)
    # 203: -(p%4==0) (bias extraction, sign folded in)
    ct[0:8, 203] = -(p8 % 4 == 0).astype(np.float32)
    # 204:332 SEL2 [8,128]: (p//4 == q//64) broadcasts per-sample w/bias
    # rows to that sample's 64 projection partitions in one matmul
    ct[0:8, 204:332] = (p8[:, None] // 4 == q[None, :] // 64)
    return ct


def _build_fused():
    import concourse.bass as bass
    import concourse.mybir as mybir

    f32 = mybir.dt.float32
    f16 = mybir.dt.float16
    alu = mybir.AluOpType
    actf = mybir.ActivationFunctionType
    act_copy = actf.Copy
    ax = mybir.AxisListType
    OO = 49

    nc = bass.Bass("TRN2", target_bir_lowering=False, debug=False,
                   num_devices=N_CORES)
    xg = nc.dram_tensor("xg", [SPC, 128, NBLK1 * BST1], f16,
                        kind="ExternalInput").ap()
    xp = nc.dram_tensor("xp", [128, OO * 4 * C], f16,
                        kind="ExternalInput").ap()
    cst = nc.dram_tensor("cst", [128, CONST_COLS], f32,
                         kind="ExternalInput").ap()
    out = nc.dram_tensor("out", [128, OO * C], f16,
                         kind="ExternalOutput").ap()
    stats = nc.dram_tensor("stats", [8, BST1], f32,
                           kind="ExternalOutput").ap()

    with _make_tile_context(nc) as tc:
        with (
            tc.tile_pool(name="cstp", bufs=1) as cstp,
            tc.tile_pool(name="inp", bufs=8) as inp,
            tc.tile_pool(name="pin", bufs=6) as pin,
            tc.tile_pool(name="psum", bufs=1, space="PSUM") as psum,
            tc.tile_pool(name="pst", bufs=1, space="PSUM") as pst,
            tc.tile_pool(name="sml", bufs=1) as sml,
            tc.tile_pool(name="eig", bufs=3) as eig,
            tc.tile_pool(name="accA", bufs=1) as accp,
            tc.tile_pool(name="accT", bufs=2) as acct,
            tc.tile_pool(name="accB", bufs=3) as accb,
        ):
            # const load on the ACT queue: ready long before first use
            ct = cstp.tile([128, CONST_COLS], f32, tag="cst")
            nc.scalar.dma_start(out=ct[:], in_=cst[:])
            mask130 = ct[:, 0:130]
            lsel0 = ct[:, 130:138]
            lsel1 = ct[:, 138:146]
            lseld = ct[0:8, 130:134]     # (p%4==l) diag selector rows 0-7
            i8 = ct[0:8, 146:154]
            m8 = ct[0:8, 154:162]
            r8 = ct[0:8, 162:163]
            p0d = ct[0:8, 163:171]
            p1d = ct[0:8, 171:179]
            i8m49 = ct[0:8, 179:187]
            m8s = ct[0:8, 187:195]
            m8n50 = ct[0:8, 195:203]
            maskp0n = ct[0:8, 203:204]
            sel2 = ct[0:8, 204:332]

            # ---- phase 1: gram matmuls over the site-block stream -------
            # (all bulk loads on the single SP queue: one queue aggregates
            # the 16 DMA engines and orders the gram stream ahead of the
            # plane stream with no gate descriptors)
            pss = []
            for s in range(SPC):
                ps = psum.tile([128, BST1], f32, tag=f"ps{s}")
                b0 = 0
                for nb in P1_TILES:
                    t = inp.tile([128, nb * BST1], f16, tag="in")
                    t3 = t[:].rearrange("p (j b) -> p j b", b=BST1)
                    nc.sync.dma_start(
                        out=t[:], in_=xg[s, :, b0 * BST1:(b0 + nb) * BST1])
                    for j in range(nb):
                        nc.tensor.matmul(ps[:, 0:BST1],
                                         t3[:, j:j + 1, 0:128],
                                         t3[:, j:j + 1, 0:BST1],
                                         start=(b0 + j == 0),
                                         stop=(b0 + j == NBLK1 - 1),
                                         skip_group_check=True)
                    b0 += nb
                pss.append(ps)

            # ---- queue all plane loads (strictly after the gram loads;
            # shared tag: tile i+6 reuses tile i's buffer, which the
            # projection has consumed long before the stream reaches it) --
            planes = []
            off = 0
            for oo in FP2_TILES:
                F = oo * C
                it = pin.tile([128, 4 * F], f16, tag="it")
                nc.sync.dma_start(out=it[:], in_=xp[:, off:off + 4 * F])
                planes.append((it, F))
                off += 4 * F

            # ---- fold: masked PSUM reads + selector matmuls ------------
            bms = []
            for s in range(SPC):
                bm = sml.tile([128, BST1], f32, tag=f"bm{s}")
                nc.vector.tensor_tensor(bm[:], pss[s][:, 0:BST1], mask130,
                                        op=alu.mult)
                bms.append(bm)
            t1p = pst.tile([8, BST1], f32, tag="t1p")
            for s in range(SPC):
                nc.tensor.matmul(t1p[:], lsel0 if s == 0 else lsel1,
                                 bms[s][:], start=(s == 0),
                                 stop=(s == SPC - 1), skip_group_check=True)
            t1b = sml.tile([8, BST1], f32, tag="t1b")
            nc.vector.tensor_copy(out=t1b[:], in_=t1p[:])
            s8 = sml.tile([8, 4], f32, tag="s8")
            nc.vector.tensor_reduce(
                s8[:], t1b[:, 0:128].rearrange("p (g l) -> p l g", l=4),
                axis=ax.X, op=alu.add)
            mu8 = sml.tile([8, 1], f32, tag="mu8")
            nc.vector.tensor_scalar(mu8[:], t1b[:, 128:129],
                                    float(1.0 / NROWS), None, op0=alu.mult)
            tmp84 = sml.tile([8, 4], f32, tag="tmp84")
            nc.vector.tensor_tensor(tmp84[:], s8[:], lseld, op=alu.mult)
            e2 = sml.tile([8, 1], f32, tag="e2")
            nc.vector.tensor_reduce(e2[:], tmp84[:], axis=ax.X, op=alu.add)
            m2 = sml.tile([8, 1], f32, tag="m2")
            nc.vector.tensor_tensor(m2[:], mu8[:], mu8[:], op=alu.mult)
            var8 = sml.tile([8, 1], f32, tag="var8")
            nc.vector.scalar_tensor_tensor(var8[:], e2[:],
                                           float(1.0 / NROWS), m2[:],
                                           op0=alu.mult, op1=alu.subtract)
            std8 = sml.tile([8, 1], f32, tag="std8")
            nc.scalar.activation(std8[:], var8[:], actf.Sqrt)
            rstd8 = sml.tile([8, 1], f32, tag="rstd8")
            nc.vector.reciprocal(rstd8[:], std8[:])

            # block-diag embed of the two 4x4 S matrices via masked PE
            # matmuls (DVE cannot address partition ranges off base 0)
            sembp = pst.tile([8, 8], f32, tag="o8")
            nc.tensor.matmul(sembp[:, 0:4], p0d, s8[:],
                             start=True, stop=True)
            nc.tensor.matmul(sembp[:, 4:8], p1d, s8[:],
                             start=True, stop=True)
            semb = sml.tile([8, 8], f32, tag="semb")
            nc.vector.tensor_copy(out=semb[:], in_=sembp[:])

            # D S D via two diag-matmuls, D = diag(1/sigma)
            dstd = sml.tile([8, 8], f32, tag="dstd")
            nc.vector.tensor_scalar(dstd[:], i8, rstd8[:], None,
                                    op0=alu.mult)
            s1p = pst.tile([8, 8], f32, tag="o8")
            nc.tensor.matmul(s1p[:], semb[:], dstd[:], start=True,
                             stop=True)
            s1c = sml.tile([8, 8], f32, tag="s1c")
            nc.vector.tensor_copy(out=s1c[:], in_=s1p[:])
            sddp = pst.tile([8, 8], f32, tag="o8")
            nc.tensor.matmul(sddp[:], dstd[:], s1c[:], start=True,
                             stop=True)
            sdd = sml.tile([8, 8], f32, tag="sdd")
            nc.vector.tensor_copy(out=sdd[:], in_=sddp[:])

            # nu nu^T outer product (nu = mu/sigma) via PE transpose
            nu = sml.tile([8, 1], f32, tag="nu")
            nc.vector.tensor_tensor(nu[:], mu8[:], rstd8[:], op=alu.mult)
            nutp = pst.tile([1, 8], f32, tag="tr")
            nc.tensor.transpose(nutp[:], nu[:], i8)
            nut = sml.tile([1, 8], f32, tag="nut")
            nc.vector.tensor_copy(out=nut[:], in_=nutp[:])
            onnp = pst.tile([8, 8], f32, tag="o8")
            nc.tensor.matmul(onnp[:], nut[:], nut[:], start=True, stop=True)
            onn = sml.tile([8, 8], f32, tag="onn")
            nc.vector.tensor_copy(out=onn[:], in_=onnp[:])

            # a8 = gram/(ESHIFT*N) - 49*I  (eigs land at 1 + 50*delta so a
            # single mid-chain fro normalization suffices)
            g8i = sml.tile([8, 8], f32, tag="g8i")
            nc.vector.scalar_tensor_tensor(g8i[:], sdd[:],
                                           float(1.0 / (ESHIFT * NROWS)),
                                           i8m49, op0=alu.mult, op1=alu.add)
            onnm = sml.tile([8, 8], f32, tag="onnm")
            nc.vector.tensor_tensor(onnm[:], onn[:], m8n50, op=alu.mult)
            a8 = eig.tile([8, 8], f32, tag="a8")
            nc.vector.tensor_tensor(a8[:], g8i[:], onnm[:], op=alu.add)

            # ---- eigensolve: repeated squaring, one per-block fro norm --
            for t in range(NSQ):
                if t == 5:
                    sq = eig.tile([8, 8], f32, tag="sq")
                    nc.vector.tensor_tensor(sq[:], a8[:], a8[:],
                                            op=alu.mult)
                    rs = eig.tile([8, 1], f32, tag="rs")
                    nc.vector.tensor_reduce(rs[:], sq[:], axis=ax.X,
                                            op=alu.add)
                    fbp = pst.tile([8, 1], f32, tag="v1")
                    nc.tensor.matmul(fbp[:], m8, rs[:], start=True,
                                     stop=True)
                    rfb = eig.tile([8, 1], f32, tag="fb")
                    nc.vector.reciprocal(rfb[:], fbp[:])
                    rfrt = eig.tile([8, 1], f32, tag="rfrt")
                    nc.scalar.activation(rfrt[:], rfb[:], actf.Sqrt)
                    an = eig.tile([8, 8], f32, tag="a8")
                    nc.vector.tensor_scalar(an[:], a8[:], rfrt[:], None,
                                            op0=alu.mult)
                    a8 = an
                a2p = pst.tile([8, 8], f32, tag="o8")
                nc.tensor.matmul(a2p[:], a8[:], a8[:], start=True,
                                 stop=True)
                a8 = eig.tile([8, 8], f32, tag="a8")
                nc.vector.tensor_copy(out=a8[:], in_=a2p[:])

            # ---- top eigenvector, w, bias ------------------------------
            v8p = pst.tile([8, 1], f32, tag="v1")
            nc.tensor.matmul(v8p[:], a8[:], r8, start=True, stop=True)
            v8 = sml.tile([8, 1], f32, tag="v8")
            nc.vector.tensor_copy(out=v8[:], in_=v8p[:])
            vsq = sml.tile([8, 1], f32, tag="vsq")
            nc.vector.tensor_tensor(vsq[:], v8[:], v8[:], op=alu.mult)
            nbp = pst.tile([8, 1], f32, tag="v1")
            nc.tensor.matmul(nbp[:], m8, vsq[:], start=True, stop=True)
            rnb = sml.tile([8, 1], f32, tag="rnb")
            nc.vector.reciprocal(rnb[:], nbp[:])
            rnrt = sml.tile([8, 1], f32, tag="rnrt")
            nc.scalar.activation(rnrt[:], rnb[:], actf.Sqrt)
            w8 = sml.tile([8, 1], f32, tag="w8")
            nc.vector.scalar_tensor_tensor(w8[:], v8[:], rnrt[:],
                                           rstd8[:], op0=alu.mult,
                                           op1=alu.mult)
            prod = sml.tile([8, 1], f32, tag="prod")
            nc.vector.tensor_tensor(prod[:], mu8[:], w8[:], op=alu.mult)
            pbp = pst.tile([8, 1], f32, tag="v1")
            nc.tensor.matmul(pbp[:], m8, prod[:], start=True, stop=True)
            pb = sml.tile([8, 1], f32, tag="pb")
            nc.vector.tensor_copy(out=pb[:], in_=pbp[:])

            # ---- broadcast w/bias to 128 partitions in ONE matmul ------
            # wb5[p,k] = w8[p]*(p%4==k), wb5[p,4] = -pb[p]*(p%4==0);
            # SEL2 sums each sample block onto its 64 partitions.
            wb5 = sml.tile([8, 5], f32, tag="wb5")
            nc.vector.tensor_scalar(wb5[:, 0:4], lseld, w8[:], None,
                                    op0=alu.mult)
            nc.vector.tensor_scalar(wb5[:, 4:5], maskp0n, pb[:], None,
                                    op0=alu.mult)
            wbp = pst.tile([128, 5], f32, tag="wbp")
            nc.tensor.matmul(wbp[:], sel2, wb5[:], start=True, stop=True)
            wball = sml.tile([128, 5], f32, tag="wball")
            nc.vector.tensor_copy(out=wball[:], in_=wbp[:])
            wl4 = wball[:, 0:4]
            bias128 = wball[:, 4:5]

            # ---- projection (pass-2 compute), split across three
            # engines so no single one carries the post-eigensolve tail:
            # DVE t0/u0/ot, GPSIMD t1/u1, ACT t2/t3 -----------------------
            w = [wl4[:, k:k + 1] for k in range(4)]
            ooff = 0
            for ti, (it, F) in enumerate(planes):
                t0 = accp.tile([128, F], f16, tag="t0")
                t1 = accp.tile([128, F], f16, tag="t1")
                t2 = acct.tile([128, F], f16, tag="t2")
                t3 = acct.tile([128, F], f16, tag="t3")
                u0 = accp.tile([128, F], f16, tag="u0")
                u1 = acct.tile([128, F], f16, tag="u1")
                ot = accb.tile([128, F], f16, tag="ot")
                nc.scalar.activation(t2[:], it[:, 2 * F:3 * F], act_copy,
                                     scale=w[2])
                nc.scalar.activation(t3[:], it[:, 3 * F:4 * F], act_copy,
                                     scale=w[3])
                nc.gpsimd.tensor_scalar(t1[:], it[:, F:2 * F], w[1], None,
                                        op0=alu.mult)
                nc.gpsimd.tensor_tensor(u1[:], t2[:], t3[:], op=alu.add)
                nc.vector.tensor_scalar(t0[:], it[:, 0:F], w[0], bias128,
                                        op0=alu.mult, op1=alu.add)
                nc.vector.tensor_tensor(u0[:], t0[:], t1[:], op=alu.add)
                nc.vector.tensor_tensor(ot[:], u0[:], u1[:], op=alu.add)
                if ti < len(planes) - 2:
                    nc.scalar.dma_start(out=out[:, ooff:ooff + F],
                                        in_=ot[:])
                else:
                    nc.sync.dma_start(out=out[:, ooff:ooff + F],
                                      in_=ot[:])
                ooff += F

            # stats ride the tail of the SP ring (host only needs them
            # after the run, for the eigenvector sign check)
            nc.sync.dma_start(out=stats[:], in_=t1b[:])
    _split_sync_waits(nc)
    return nc


def _get_programs():
    global _programs
    if _programs is None:
        _programs = (_build_pass1(), _build_pass2())
    return _programs


def _get_fused():
    global _fused_program
    if _fused_program is None:
        _fused_program = _build_fused()
    return _fused_program


def _host_fold(stats8):
    """stats8: [B//SPC, 8, 130] f32 device-folded stats -> per-sample
    mu [B,4], sigma [B,4], comp [B,4] (reference-sign top eigenvector).

    Row 4s+m of a core's [8, 130] block holds M_s[4*(col//4)+m, col] for
    col<128 (exact copies of the block-diagonal entries) and colsum_s[m]
    at col 128.  Downstream matches the reference exactly: gram from
    (S - N mu mu^T) / (sigma sigma^T), comp = eigh(gram f32) top
    eigenvector on CPU jax.
    """
    st = stats8.astype(np.float64).reshape(B, 4, BST1)   # [b, m, col]
    t1 = st[:, :, :128].reshape(B, 4, 32, 4)             # [b, k, g, l]
    S = t1.sum(axis=2)                                   # [B, 4, 4]
    colsum = st[:, :, 128]                               # [B, 4]

    mu = colsum / NROWS
    e2 = np.einsum("bkk->bk", S) / NROWS
    var = np.maximum(e2 - mu * mu, 0.0)
    sigma = np.sqrt(var)
    denom = sigma[:, :, None] * sigma[:, None, :]
    gram = (S - NROWS * mu[:, :, None] * mu[:, None, :])
    with np.errstate(divide="ignore", invalid="ignore"):
        gram = np.where(denom > 0, gram / np.where(denom > 0, denom, 1.0), 0.0)

    # eigh with the same implementation/backend the reference uses (CPU jax)
    import jax
    import jax.numpy as jnp
    with jax.default_device(jax.devices("cpu")[0]):
        V = np.asarray(jnp.linalg.eigh(jnp.asarray(gram, jnp.float32))[1])
    comp = V[:, :, -1].astype(np.float64)                # top eigenvector
    return mu, sigma, comp


def _host_comp(stats8):
    return _host_fold(stats8)[2]


def _host_middle(stats8):
    """stats8 -> w [B, 4] f64, bias [B] f64 for the host-prescale path."""
    mu, sigma, comp = _host_fold(stats8)
    with np.errstate(divide="ignore", invalid="ignore"):
        w = np.where(sigma > 0, comp / np.where(sigma > 0, sigma, 1.0), 0.0)
    bias = -(mu * w).sum(axis=1)
    return w, bias


def _layouts(x16):
    """Build the gram-pass and plane-pass device layouts from fp16 x."""
    xp = np.zeros((B, 128, NBLK1, BST1), np.float16)
    xp[..., :128] = x16.reshape(B, NBLK1, 128, 128).transpose(0, 2, 1, 3)
    xp[..., 128] = 1.0
    xp = xp.reshape(B, 128, NBLK1 * BST1)

    xpl = x16.reshape(B, HO, 2, WO, 2, C // 4, 4).transpose(
        0, 1, 3, 6, 2, 4, 5)
    xpl = np.ascontiguousarray(xpl).reshape(B, 64, 49, 4, C)
    segs = []
    oo0 = 0
    for oo in FP2_TILES:
        seg = xpl[:, :, oo0:oo0 + oo].transpose(0, 1, 3, 2, 4)
        segs.append(seg.reshape(B, 64, 4 * oo * C))
        oo0 += oo
    x2h = np.concatenate(segs, axis=2)             # [B, 64, 49*4*C]
    return xp, x2h


def _layout_p2_scaled(x16, w, bias):
    """k-plane pass-2 layout with w_k folded into each plane and the bias
    folded into plane 0 (host knows w after pass 1; the scale rides the
    layout pass that exists anyway).  Returns x2h [B, 64, 49*4*C] fp16."""
    xpl = x16.reshape(B, HO, 2, WO, 2, C // 4, 4).transpose(
        0, 1, 3, 6, 2, 4, 5)
    xpl = np.ascontiguousarray(xpl).reshape(B, 3136, 4, C)
    xs = xpl.astype(np.float32)
    xs *= w.astype(np.float32)[:, None, :, None]
    xs[:, :, 0, :] += bias.astype(np.float32)[:, None, None]
    xpl = xs.astype(np.float16).reshape(B, 64, 49, 4, C)
    segs = []
    oo0 = 0
    for oo in P2_TILES:
        seg = xpl[:, :, oo0:oo0 + oo].transpose(0, 1, 3, 2, 4)
        segs.append(seg.reshape(B, 64, 4 * oo * C))
        oo0 += oo
    return np.concatenate(segs, axis=2)             # [B, 64, 49*4*C]


def _kernel_fused(x16):
    from concourse.bass_utils import run_bass_kernel_spmd

    ncf = _get_fused()
    core_ids = list(range(N_CORES))
    xp, x2h = _layouts(x16)
    cst = _host_consts()
    ins = []
    for c in range(N_CORES):
        pair = x2h[c * SPC:(c + 1) * SPC]
        ins.append({
            "xg": xp[c * SPC:(c + 1) * SPC],
            "xp": pair.reshape(128, 49 * 4 * C),
            "cst": cst,
        })
    kw = dict(trace=True, tmpdir=TRACE_DIRS.get("pass1")) if TRACE else {}
    r = run_bass_kernel_spmd(ncf, ins, core_ids, **kw)
    if TRACE:
        LAST_PROFILE["pass1_ns"] = r.exec_time_ns
        LAST_PROFILE["pass2_ns"] = 0

    # Sign fix: the device's power iteration returns comp * sign(comp.r8)
    # (even power of a positive-top-eig matrix applied to the fixed probe
    # r8), while the reference's eigh sign is whatever LAPACK produced.
    # comp_host from the device-folded stats tells us both.
    stats8 = np.stack([r.results[c]["stats"] for c in range(N_CORES)])
    comp = _host_comp(stats8)                     # [B, 4] reference-sign
    rp = np.array(R_PROBE, np.float64)
    flip = (comp @ rp) < 0                        # device sign != host sign

    outs = []
    for c in range(N_CORES):
        o = r.results[c]["out"].astype(np.float32).reshape(SPC, HO, WO, C)
        for s in range(SPC):
            if flip[c * SPC + s]:
                o[s] = -o[s]
        outs.append(o)
    return np.ascontiguousarray(np.concatenate(outs))


def kernel(x):
    from concourse.bass_utils import run_bass_kernel_spmd

    x = np.asarray(x)
    assert x.shape == (B, H, W, C), x.shape
    x16 = np.ascontiguousarray(x, dtype=np.float16)
    if FUSED:
        return _kernel_fused(x16)
    nc1, nc2 = _get_programs()
    core_ids = list(range(N_CORES))

    # pass-1 input: 128-row x 128-col blocks padded to 130 cols with a
    # ones column at 128 ([128 partitions, blocks]); row r = pix*2 + half
    xp = np.zeros((B, 128, NBLK1, BST1), np.float16)
    xp[..., :128] = x16.reshape(B, NBLK1, 128, 128).transpose(0, 2, 1, 3)
    xp[..., 128] = 1.0
    xp = xp.reshape(B, 128, NBLK1 * BST1)
    cst1 = _p1_consts()
    in1 = [{"x": xp[c * SPC:(c + 1) * SPC], "cst": cst1}
           for c in range(N_CORES)]
    kw1 = dict(trace=True, tmpdir=TRACE_DIRS.get("pass1")) if TRACE else {}
    r1 = run_bass_kernel_spmd(nc1, in1, core_ids, **kw1)
    if TRACE:
        LAST_PROFILE["pass1_ns"] = r1.exec_time_ns
    stats8 = np.stack([r1.results[c]["stats"] for c in range(N_CORES)])

    w, bias = _host_middle(stats8)
    x2h = _layout_p2_scaled(x16, w, bias)
    in2 = []
    for c in range(N_CORES):
        pair = x2h[c * SPC:(c + 1) * SPC]          # [2, 64, 49*4*C]
        in2.append({"x": pair.reshape(128, 49 * 4 * C)})
    kw2 = dict(trace=True, tmpdir=TRACE_DIRS.get("pass2")) if TRACE else {}
    r2 = run_bass_kernel_spmd(nc2, in2, core_ids, **kw2)
    if TRACE:
        LAST_PROFILE["pass2_ns"] = r2.exec_time_ns

    # gather: out[s*64+p64, oo*C+c'], outpix = p64*49+oo -> [B, HO, WO, C]
    outs = [r2.results[c]["out"].astype(np.float32).reshape(SPC, HO, WO, C)
            for c in range(N_CORES)]
    return np.ascontiguousarray(np.concatenate(outs))



# revision 29
# speedup vs baseline: 2.5041x; 2.5041x over previous
"""BPCA pooling layer on 8 Trainium2 NeuronCores (Bass/Tile).

Math: per sample, the reference's `data = patches.reshape(-1, 4)` groups 4
consecutive channels (C=256 is divisible by 4), so `data` is exactly the
sample's contiguous buffer viewed as [N, 4] with N = H*W*C/4.  The layer is:

  1. per-column mean/std over N rows, dn = (data-mean)/std
  2. gram = dn^T dn (4x4), comp = top eigenvector (jnp.linalg.eigh)
  3. out = (dn @ comp) reshaped to [H/2, W/2, C] with channel permutation
     c' = (2*di+dj)*64 + (c//4)

Device plan (2 samples per core, pure data parallel).  Both passes are
DMA-bound, so all device I/O except the tiny stats tensor is fp16 --
quantizing x to fp16 perturbs the final output by ~3e-4 rel (measured
against the fixed seed), 60x under the 2e-2 gate, and halves traffic:

  pass 1: PE computes the 128x128 half-channel second-moment matrix
          M[j,j'] = sum_{pix,G} x[pix,128G+j]*x[pix,128G+j'] plus column
          sums (ones column), accumulated in fp32 PSUM, from fp16 inputs.
          128-col blocks (one matmul per 128-row block, N=130 moving)
          instead of 256-col halves: same LDWEIGHTS count but half the
          moving columns, so the PE stream (~81ns/MM warm) stays under
          the DMA stream.
  host:   fold M into the 4x4 gram (S_kl = sum_g M[4g+k,4g+l]), compute
          mean/std/gram in f64, eigh on CPU jax (same implementation the
          reference uses), derive w_k = comp_k/std_k and
          bias = -sum_k mean_k*comp_k/std_k.
  pass 2: out = sum of four host-prescaled k-planes (the host builds the
          k-plane layout after pass 1, when it already has w/bias, so the
          per-plane scale w_k and the bias ride that existing layout
          pass) -- three tensor_tensor adds on DVE (2x mode for packed
          fp16), far under the DMA stream.  NOT a scalar_tensor_tensor
          chain: STT has no accelerated DVE uops and runs 1 elem/cyc.
          Output channel permutation is folded into the host layout.

All bulk loads go through the single SP DMA queue: one queue aggregates
~350 GB/s across the 16 DMA engines, while splitting across two queues
measured LOWER total (the engines, not the queue, are the resource);
the ACT queue only carries small stores (it gets poor engine service).
Graduated tile ladders (small head, small tail) plus bufs=8 prefetch
keep the whole-tile DMA-completion semaphores off the critical path.
"""

import numpy as np

# ---------------------------------------------------------------------------
# Problem constants (hardcoded per spec)
# ---------------------------------------------------------------------------
B, H, W, C = 16, 112, 112, 256
N_CORES = 8
SPC = B // N_CORES          # samples per core = 2
PIX = H * W                 # 12544 pixels per sample
NBLK1 = PIX * C // (128 * 128)  # 196 row-blocks of 128 per sample
BST1 = 130                  # per-block SBUF cols: 128 data + 1 ones + 1 pad
P1_TILES = [32, 32, 32, 32, 32, 24, 12]  # (sum=196)
# uniform 32-block tiles keep 8.3KB per-partition DMA segments through the
# stream (the old graduated head ran 0.5-5KB segments and measurably
# dropped the stream rate); the shrinking tail bounds the PE chase after
# the last tile lands
NROWS = PIX * C // 4        # 802816 rows of the [N, 4] data matrix
HO, WO = H // 2, W // 2     # 56 x 56 output

_programs = None
_fused_program = None
LAST_PROFILE = {}
TRACE = False
TRACE_DIRS = {}
FUSED = True                # single NEFF: pays the ~9us queue-startup head
                            # and the ~2.3us end fence once instead of twice
NSQ = 9                     # matrix squarings (power 512; sim err 8e-5)
P2_TILES = [2, 4, 8, 12, 12, 9, 2]
FP2_TILES = [7] * 7         # fused plane tiles: uniform 14.3KB segments
CONST_COLS = 332
P1_CONST_COLS = 146


# ---------------------------------------------------------------------------
# TileContext with a walrus-compatible tail drain
# ---------------------------------------------------------------------------
def _make_tile_context(nc):
    from concourse.tile import TileContext
    return TileContext(nc)


def _split_sync_waits(nc):
    """walrus (CoreV2/V3 codegen) rejects instructions carrying more than 2
    sync commands (waits + updates combined); Tile freely emits e.g. 2 waits
    + 1 update.  Hoist excess waits onto same-engine NOPs inserted directly
    before the offending instruction -- same engine means the same program-
    order point, so semantics are unchanged."""
    import concourse.mybir as mybir

    def mint_nop(engine):
        inner = nc.engines[engine].nop().ins
        for blk in nc.m.functions[0].blocks:
            il = blk.instructions
            for k in range(len(il) - 1, -1, -1):
                if il[k] is inner:
                    il.pop(k)
                    return inner
        raise RuntimeError("minted nop not found in any block")

    for fn in nc.m.functions:
        for blk in fn.blocks:
            il = blk.instructions
            i = 0
            while i < len(il):
                inst = il[i]
                si = inst.sync_info
                waits = list(si.on_wait) if si and si.on_wait else []
                upds = list(si.on_update) if si and si.on_update else []
                # observed walrus limits: at most 1 wait per instruction
                # (1 wait + 1 update compiles; 2 waits anywhere does not)
                if len(waits) > 1:
                    extra, keep = waits[:-1], waits[-1:]
                    for wchunk in extra:
                        nop = mint_nop(inst.engine)
                        nop.sync_info = mybir.SyncInfo(
                            on_wait=[wchunk], on_update=[])
                        il.insert(i, nop)
                        i += 1
                    inst.sync_info = mybir.SyncInfo(
                        on_wait=keep, on_update=upds)
                i += 1


def _p1_consts():
    """[128, 146] f32: cols 0:130 block-diag fold mask (col 128 = ones for
    the chansum column, col 129 = 0 to kill the pad col), 130:138 sample-0
    row selector (p%4==m in col 130+m), 138:146 sample-1 selector (col
    142+m)."""
    ct = np.zeros((128, P1_CONST_COLS), np.float32)
    p = np.arange(128)
    q = np.arange(128)
    ct[:, 0:128] = (p[:, None] // 4 == q[None, :] // 4).astype(np.float32)
    ct[:, 128] = 1.0
    for m in range(4):
        ct[:, 130 + m] = (p % 4 == m)
        ct[:, 142 + m] = (p % 4 == m)
    return ct


def _build_pass1():
    import concourse.bass as bass
    import concourse.mybir as mybir

    f32 = mybir.dt.float32
    f16 = mybir.dt.float16
    alu = mybir.AluOpType

    nc = bass.Bass("TRN2", target_bir_lowering=False, debug=False,
                   num_devices=N_CORES)
    # The host pre-interleaves a ones column per block (col 128 of each
    # 130-wide block) so one DMA loads data + ones and no on-device memset
    # is needed.
    x = nc.dram_tensor("x", [SPC, 128, NBLK1 * BST1], f16,
                       kind="ExternalInput").ap()
    cst = nc.dram_tensor("cst", [128, P1_CONST_COLS], f32,
                         kind="ExternalInput").ap()
    # folded stats: rows 4s+m hold M_s[4*(col//4)+m, col] for col<128 (an
    # exact selector-matmul copy of the block-diag entries the host fold
    # uses) and colsum_s[m] in col 128.  4KB store instead of the old
    # 2x[128,130] (255KB) whose 288-packet drain on the scalar queue was
    # ~5us of critical-path tail.
    stats = nc.dram_tensor("stats", [8, BST1], f32,
                           kind="ExternalOutput").ap()

    with _make_tile_context(nc) as tc:
        with (
            tc.tile_pool(name="cstp", bufs=1) as cstp,
            tc.tile_pool(name="inp", bufs=8) as inp,
            tc.tile_pool(name="psum", bufs=2, space="PSUM") as psum,
            tc.tile_pool(name="pst", bufs=1, space="PSUM") as pst,
            tc.tile_pool(name="sml", bufs=1) as sml,
        ):
            # const load on the ACT queue: the load queue stays clean and
            # the transfer (75KB) completes long before the first fold use
            ct = cstp.tile([128, P1_CONST_COLS], f32, tag="cst")
            nc.scalar.dma_start(out=ct[:], in_=cst[:])
            mask130 = ct[:, 0:130]
            lsels = [ct[:, 130:138], ct[:, 138:146]]

            bms = []
            for s in range(SPC):
                ps = psum.tile([128, BST1], f32, tag="ps")
                b0 = 0
                for nb in P1_TILES:
                    t = inp.tile([128, nb * BST1], f16, tag="in")
                    t3 = t[:].rearrange("p (j b) -> p j b", b=BST1)
                    nc.sync.dma_start(
                        out=t[:],
                        in_=x[s, :, b0 * BST1:(b0 + nb) * BST1])
                    for j in range(nb):
                        first = b0 + j == 0
                        last = b0 + j == NBLK1 - 1
                        nc.tensor.matmul(ps[:, 0:BST1],
                                         t3[:, j:j + 1, 0:128],
                                         t3[:, j:j + 1, 0:BST1],
                                         start=first, stop=last,
                                         skip_group_check=True)
                    b0 += nb
                # block-diag mask applied straight from PSUM (sample 0's
                # runs mid-stream on the idle DVE)
                bm = sml.tile([128, BST1], f32, tag=f"bm{s}")
                nc.vector.tensor_tensor(bm[:], ps[:, 0:BST1], mask130,
                                        op=alu.mult)
                bms.append(bm)
            # both selector matmuls at the end so the PE gram streams are
            # never interrupted by a wait on the DVE mask-mult
            t1p = pst.tile([8, BST1], f32, tag="t1p")
            for s in range(SPC):
                nc.tensor.matmul(t1p[:], lsels[s], bms[s][:],
                                 start=(s == 0), stop=(s == SPC - 1),
                                 skip_group_check=True)
            t1b = sml.tile([8, BST1], f32, tag="t1b")
            nc.vector.tensor_copy(out=t1b[:], in_=t1p[:])
            # the load queue is idle once the last load retires, so this
            # drains immediately (the ACT queue adds ~3us of service lag)
            nc.sync.dma_start(out=stats[:], in_=t1b[:])
    _split_sync_waits(nc)
    return nc


def _build_pass2():
    import concourse.bass as bass
    import concourse.mybir as mybir

    f16 = mybir.dt.float16
    alu = mybir.AluOpType
    OO = 49  # output pixels per partition (3136 = 64 partitions x 49)

    nc = bass.Bass("TRN2", target_bir_lowering=False, debug=False,
                   num_devices=N_CORES)
    # Partition p = (s_local*64 + p64): both samples fill 128 partitions so
    # every DMA is a fully-contiguous 128-partition transfer.  The host
    # builds the k-plane layout AFTER pass 1 (it already has w/bias then),
    # so the per-plane scale w_k and the bias (folded into plane 0) ride
    # the existing host layout pass; the device sums the four planes with
    # three tensor_tensor adds (2x DVE mode for packed fp16).
    x = nc.dram_tensor("x", [128, OO * 4 * C], f16,
                       kind="ExternalInput").ap()
    out = nc.dram_tensor("out", [128, OO * C], f16,
                         kind="ExternalOutput").ap()

    with _make_tile_context(nc) as tc:
        with (
            tc.tile_pool(name="inp", bufs=4) as inp,
            tc.tile_pool(name="acc", bufs=2) as accp,
            tc.tile_pool(name="otp", bufs=4) as otp,
        ):
            off = 0
            ooff = 0
            for ti, oo in enumerate(P2_TILES):
                F = oo * C
                it = inp.tile([128, 4 * F], f16, tag="it")
                nc.sync.dma_start(out=it[:], in_=x[:, off:off + 4 * F])
                u0 = accp.tile([128, F], f16, tag="u0")
                u1 = accp.tile([128, F], f16, tag="u1")
                # deep ot pool: DVE never waits on ACT-queue store service
                ot = otp.tile([128, F], f16, tag="ot")
                nc.vector.tensor_tensor(
                    u0[:], it[:, 0:F], it[:, F:2 * F], op=alu.add)
                nc.vector.tensor_tensor(
                    u1[:], it[:, 2 * F:3 * F], it[:, 3 * F:4 * F],
                    op=alu.add)
                nc.vector.tensor_tensor(
                    ot[:], u0[:], u1[:], op=alu.add)
                if ti < len(P2_TILES) - 2:
                    # mid-stream stores on the ACT queue: the load queue
                    # stays free to prefetch
                    nc.scalar.dma_start(
                        out=out[:, ooff:ooff + F], in_=ot[:])
                else:
                    # the last stores ride the load queue, which is idle
                    # once the final load descriptor retires and drains
                    # far faster than the ACT queue
                    nc.sync.dma_start(
                        out=out[:, ooff:ooff + F], in_=ot[:])
                off += 4 * F
                ooff += F
    _split_sync_waits(nc)
    return nc


R_PROBE = (0.5393, -0.2117, 0.8313, 0.1078)  # fixed eig probe (per k)
ESHIFT = 0.02                                # a8 = gram/(ESHIFT*N) - 49*I


def _host_consts():
    """Constant tensor for the fused kernel's on-device fold/eigensolve."""
    ct = np.zeros((128, CONST_COLS), np.float32)
    p = np.arange(128)
    q = np.arange(128)
    p8 = np.arange(8)
    # 0:130 block-diag mask for M -> per-group fold (col 128 = chansums)
    ct[:, 0:128] = (p[:, None] // 4 == q[None, :] // 4).astype(np.float32)
    ct[:, 128] = 1.0
    # 130:138 / 138:146 per-sample fold selectors (PSUM-accumulated);
    # rows 0:8 of 130:134 double as lseld (p%4==l diag selector)
    for m in range(4):
        ct[:, 130 + m] = (p % 4 == m)
        ct[:, 142 + m] = (p % 4 == m)
    # 146:154 I8, 154:162 blockones8
    ct[0:8, 146:154] = np.eye(8, dtype=np.float32)
    ct[0:8, 154:162] = (p8[:, None] // 4 == p8[None, :] // 4)
    # 162 r8 (fixed probe vector, repeated per sample)
    ct[0:8, 162] = np.array(R_PROBE, np.float32)[p8 % 4]
    # 163:171 / 171:179 half-diagonal projectors (block-diag embed of S)
    ct[0:8, 163:171] = np.diag((p8 < 4).astype(np.float32))
    ct[0:8, 171:179] = np.diag((p8 >= 4).astype(np.float32))
    # 179:187 -49*I8; 187:195 m8/(ESHIFT*N); 195:203 -m8/ESHIFT
    ct[0:8, 179:187] = -49.0 * np.eye(8, dtype=np.float32)
    ct[0:8, 187:195] = ct[0:8, 154:162] / (ESHIFT * NROWS)
    ct[0:8, 195:203] = ct[0:8, 154:162] * (-1.0 / ESHIFT)
    # 203: -(p%4==0) (bias extraction, sign folded in)
    ct[0:8, 203] = -(p8 % 4 == 0).astype(np.float32)
    # 204:332 SEL2 [8,128]: (p//4 == q//64) broadcasts per-sample w/bias
    # rows to that sample's 64 projection partitions in one matmul
    ct[0:8, 204:332] = (p8[:, None] // 4 == q[None, :] // 64)
    return ct


def _build_fused():
    import concourse.bass as bass
    import concourse.mybir as mybir

    f32 = mybir.dt.float32
    f16 = mybir.dt.float16
    alu = mybir.AluOpType
    actf = mybir.ActivationFunctionType
    act_copy = actf.Copy
    ax = mybir.AxisListType
    OO = 49

    nc = bass.Bass("TRN2", target_bir_lowering=False, debug=False,
                   num_devices=N_CORES)
    xg = nc.dram_tensor("xg", [SPC, 128, NBLK1 * BST1], f16,
                        kind="ExternalInput").ap()
    xp = nc.dram_tensor("xp", [128, OO * 4 * C], f16,
                        kind="ExternalInput").ap()
    cst = nc.dram_tensor("cst", [128, CONST_COLS], f32,
                         kind="ExternalInput").ap()
    out = nc.dram_tensor("out", [128, OO * C], f16,
                         kind="ExternalOutput").ap()
    stats = nc.dram_tensor("stats", [8, BST1], f32,
                           kind="ExternalOutput").ap()

    with _make_tile_context(nc) as tc:
        with (
            tc.tile_pool(name="cstp", bufs=1) as cstp,
            tc.tile_pool(name="inp", bufs=8) as inp,
            tc.tile_pool(name="pin", bufs=6) as pin,
            tc.tile_pool(name="psum", bufs=1, space="PSUM") as psum,
            tc.tile_pool(name="pst", bufs=1, space="PSUM") as pst,
            tc.tile_pool(name="sml", bufs=1) as sml,
            tc.tile_pool(name="eig", bufs=3) as eig,
            tc.tile_pool(name="accA", bufs=1) as accp,
            tc.tile_pool(name="accT", bufs=2) as acct,
            tc.tile_pool(name="accB", bufs=3) as accb,
        ):
            # const load on the ACT queue: ready long before first use
            ct = cstp.tile([128, CONST_COLS], f32, tag="cst")
            nc.scalar.dma_start(out=ct[:], in_=cst[:])
            mask130 = ct[:, 0:130]
            lsel0 = ct[:, 130:138]
            lsel1 = ct[:, 138:146]
            lseld = ct[0:8, 130:134]     # (p%4==l) diag selector rows 0-7
            i8 = ct[0:8, 146:154]
            m8 = ct[0:8, 154:162]
            r8 = ct[0:8, 162:163]
            p0d = ct[0:8, 163:171]
            p1d = ct[0:8, 171:179]
            i8m49 = ct[0:8, 179:187]
            m8s = ct[0:8, 187:195]
            m8n50 = ct[0:8, 195:203]
            maskp0n = ct[0:8, 203:204]
            sel2 = ct[0:8, 204:332]

            # ---- phase 1: gram matmuls over the site-block stream -------
            # (all bulk loads on the single SP queue: one queue aggregates
            # the 16 DMA engines and orders the gram stream ahead of the
            # plane stream with no gate descriptors)
            pss = []
            for s in range(SPC):
                ps = psum.tile([128, BST1], f32, tag=f"ps{s}")
                b0 = 0
                for nb in P1_TILES:
                    t = inp.tile([128, nb * BST1], f16, tag="in")
                    t3 = t[:].rearrange("p (j b) -> p j b", b=BST1)
                    nc.sync.dma_start(
                        out=t[:], in_=xg[s, :, b0 * BST1:(b0 + nb) * BST1])
                    for j in range(nb):
                        nc.tensor.matmul(ps[:, 0:BST1],
                                         t3[:, j:j + 1, 0:128],
                                         t3[:, j:j + 1, 0:BST1],
                                         start=(b0 + j == 0),
                                         stop=(b0 + j == NBLK1 - 1),
                                         skip_group_check=True)
                    b0 += nb
                pss.append(ps)

            # ---- queue all plane loads (strictly after the gram loads;
            # shared tag: tile i+6 reuses tile i's buffer, which the
            # projection has consumed long before the stream reaches it) --
            planes = []
            off = 0
            for oo in FP2_TILES:
                F = oo * C
                it = pin.tile([128, 4 * F], f16, tag="it")
                nc.sync.dma_start(out=it[:], in_=xp[:, off:off + 4 * F])
                planes.append((it, F))
                off += 4 * F

            # ---- fold: masked PSUM reads + selector matmuls ------------
            bms = []
            for s in range(SPC):
                bm = sml.tile([128, BST1], f32, tag=f"bm{s}")
                nc.vector.tensor_tensor(bm[:], pss[s][:, 0:BST1], mask130,
                                        op=alu.mult)
                bms.append(bm)
            t1p = pst.tile([8, BST1], f32, tag="t1p")
            for s in range(SPC):
                nc.tensor.matmul(t1p[:], lsel0 if s == 0 else lsel1,
                                 bms[s][:], start=(s == 0),
                                 stop=(s == SPC - 1), skip_group_check=True)
            t1b = sml.tile([8, BST1], f32, tag="t1b")
            nc.vector.tensor_copy(out=t1b[:], in_=t1p[:])
            s8 = sml.tile([8, 4], f32, tag="s8")
            nc.vector.tensor_reduce(
                s8[:], t1b[:, 0:128].rearrange("p (g l) -> p l g", l=4),
                axis=ax.X, op=alu.add)
            mu8 = sml.tile([8, 1], f32, tag="mu8")
            nc.vector.tensor_scalar(mu8[:], t1b[:, 128:129],
                                    float(1.0 / NROWS), None, op0=alu.mult)
            tmp84 = sml.tile([8, 4], f32, tag="tmp84")
            nc.vector.tensor_tensor(tmp84[:], s8[:], lseld, op=alu.mult)
            e2 = sml.tile([8, 1], f32, tag="e2")
            nc.vector.tensor_reduce(e2[:], tmp84[:], axis=ax.X, op=alu.add)
            m2 = sml.tile([8, 1], f32, tag="m2")
            nc.vector.tensor_tensor(m2[:], mu8[:], mu8[:], op=alu.mult)
            var8 = sml.tile([8, 1], f32, tag="var8")
            nc.vector.scalar_tensor_tensor(var8[:], e2[:],
                                           float(1.0 / NROWS), m2[:],
                                           op0=alu.mult, op1=alu.subtract)
            std8 = sml.tile([8, 1], f32, tag="std8")
            nc.scalar.activation(std8[:], var8[:], actf.Sqrt)
            rstd8 = sml.tile([8, 1], f32, tag="rstd8")
            nc.vector.reciprocal(rstd8[:], std8[:])

            # block-diag embed of the two 4x4 S matrices via masked PE
            # matmuls (DVE cannot address partition ranges off base 0)
            sembp = pst.tile([8, 8], f32, tag="o8")
            nc.tensor.matmul(sembp[:, 0:4], p0d, s8[:],
                             start=True, stop=True)
            nc.tensor.matmul(sembp[:, 4:8], p1d, s8[:],
                             start=True, stop=True)
            semb = sml.tile([8, 8], f32, tag="semb")
            nc.vector.tensor_copy(out=semb[:], in_=sembp[:])

            # D S D via two diag-matmuls, D = diag(1/sigma)
            dstd = sml.tile([8, 8], f32, tag="dstd")
            nc.vector.tensor_scalar(dstd[:], i8, rstd8[:], None,
                                    op0=alu.mult)
            s1p = pst.tile([8, 8], f32, tag="o8")
            nc.tensor.matmul(s1p[:], semb[:], dstd[:], start=True,
                             stop=True)
            s1c = sml.tile([8, 8], f32, tag="s1c")
            nc.vector.tensor_copy(out=s1c[:], in_=s1p[:])
            sddp = pst.tile([8, 8], f32, tag="o8")
            nc.tensor.matmul(sddp[:], dstd[:], s1c[:], start=True,
                             stop=True)
            sdd = sml.tile([8, 8], f32, tag="sdd")
            nc.vector.tensor_copy(out=sdd[:], in_=sddp[:])

            # nu nu^T outer product (nu = mu/sigma) via PE transpose
            nu = sml.tile([8, 1], f32, tag="nu")
            nc.vector.tensor_tensor(nu[:], mu8[:], rstd8[:], op=alu.mult)
            nutp = pst.tile([1, 8], f32, tag="tr")
            nc.tensor.transpose(nutp[:], nu[:], i8)
            nut = sml.tile([1, 8], f32, tag="nut")
            nc.vector.tensor_copy(out=nut[:], in_=nutp[:])
            onnp = pst.tile([8, 8], f32, tag="o8")
            nc.tensor.matmul(onnp[:], nut[:], nut[:], start=True, stop=True)
            onn = sml.tile([8, 8], f32, tag="onn")
            nc.vector.tensor_copy(out=onn[:], in_=onnp[:])

            # a8 = gram/(ESHIFT*N) - 49*I  (eigs land at 1 + 50*delta so a
            # single mid-chain fro normalization suffices)
            g8i = sml.tile([8, 8], f32, tag="g8i")
            nc.vector.scalar_tensor_tensor(g8i[:], sdd[:],
                                           float(1.0 / (ESHIFT * NROWS)),
                                           i8m49, op0=alu.mult, op1=alu.add)
            onnm = sml.tile([8, 8], f32, tag="onnm")
            nc.vector.tensor_tensor(onnm[:], onn[:], m8n50, op=alu.mult)
            a8 = eig.tile([8, 8], f32, tag="a8")
            nc.vector.tensor_tensor(a8[:], g8i[:], onnm[:], op=alu.add)

            # ---- eigensolve: repeated squaring, one per-block fro norm --
            for t in range(NSQ):
                if t == 5:
                    sq = eig.tile([8, 8], f32, tag="sq")
                    nc.vector.tensor_tensor(sq[:], a8[:], a8[:],
                                            op=alu.mult)
                    rs = eig.tile([8, 1], f32, tag="rs")
                    nc.vector.tensor_reduce(rs[:], sq[:], axis=ax.X,
                                            op=alu.add)
                    fbp = pst.tile([8, 1], f32, tag="v1")
                    nc.tensor.matmul(fbp[:], m8, rs[:], start=True,
                                     stop=True)
                    rfb = eig.tile([8, 1], f32, tag="fb")
                    nc.vector.reciprocal(rfb[:], fbp[:])
                    rfrt = eig.tile([8, 1], f32, tag="rfrt")
                    nc.scalar.activation(rfrt[:], rfb[:], actf.Sqrt)
                    an = eig.tile([8, 8], f32, tag="a8")
                    nc.vector.tensor_scalar(an[:], a8[:], rfrt[:], None,
                                            op0=alu.mult)
                    a8 = an
                a2p = pst.tile([8, 8], f32, tag="o8")
                nc.tensor.matmul(a2p[:], a8[:], a8[:], start=True,
                                 stop=True)
                a8 = eig.tile([8, 8], f32, tag="a8")
                nc.vector.tensor_copy(out=a8[:], in_=a2p[:])

            # ---- top eigenvector, w, bias ------------------------------
            v8p = pst.tile([8, 1], f32, tag="v1")
            nc.tensor.matmul(v8p[:], a8[:], r8, start=True, stop=True)
            v8 = sml.tile([8, 1], f32, tag="v8")
            nc.vector.tensor_copy(out=v8[:], in_=v8p[:])
            vsq = sml.tile([8, 1], f32, tag="vsq")
            nc.vector.tensor_tensor(vsq[:], v8[:], v8[:], op=alu.mult)
            nbp = pst.tile([8, 1], f32, tag="v1")
            nc.tensor.matmul(nbp[:], m8, vsq[:], start=True, stop=True)
            rnb = sml.tile([8, 1], f32, tag="rnb")
            nc.vector.reciprocal(rnb[:], nbp[:])
            rnrt = sml.tile([8, 1], f32, tag="rnrt")
            nc.scalar.activation(rnrt[:], rnb[:], actf.Sqrt)
            w8 = sml.tile([8, 1], f32, tag="w8")
            nc.vector.scalar_tensor_tensor(w8[:], v8[:], rnrt[:],
                                           rstd8[:], op0=alu.mult,
                                           op1=alu.mult)
            prod = sml.tile([8, 1], f32, tag="prod")
            nc.vector.tensor_tensor(prod[:], mu8[:], w8[:], op=alu.mult)
            pbp = pst.tile([8, 1], f32, tag="v1")
            nc.tensor.matmul(pbp[:], m8, prod[:], start=True, stop=True)
            pb = sml.tile([8, 1], f32, tag="pb")
            nc.vector.tensor_copy(out=pb[:], in_=pbp[:])

            # ---- broadcast w/bias to 128 partitions in ONE matmul ------
            # wb5[p,k] = w8[p]*(p%4==k), wb5[p,4] = -pb[p]*(p%4==0);
            # SEL2 sums each sample block onto its 64 partitions.
            wb5 = sml.tile([8, 5], f32, tag="wb5")
            nc.vector.tensor_scalar(wb5[:, 0:4], lseld, w8[:], None,
                                    op0=alu.mult)
            nc.vector.tensor_scalar(wb5[:, 4:5], maskp0n, pb[:], None,
                                    op0=alu.mult)
            wbp = pst.tile([128, 5], f32, tag="wbp")
            nc.tensor.matmul(wbp[:], sel2, wb5[:], start=True, stop=True)
            wball = sml.tile([128, 5], f32, tag="wball")
            nc.vector.tensor_copy(out=wball[:], in_=wbp[:])
            wl4 = wball[:, 0:4]
            bias128 = wball[:, 4:5]

            # ---- projection (pass-2 compute) ---------------------------
            # DVE TS runs ~4.6 el/cyc and TT ~3.3 el/cyc (4x perf mode);
            # ACT takes one scale (~1.1 el/cyc) so neither engine carries
            # the whole post-eigensolve tail.  (gpsimd tensor ops are
            # ~30x slower AND their SBUF traffic starves the DVE - never.)
            w = [wl4[:, k:k + 1] for k in range(4)]
            ooff = 0
            for ti, (it, F) in enumerate(planes):
                t0 = accp.tile([128, F], f16, tag="t0")
                t1 = accp.tile([128, F], f16, tag="t1")
                t2 = acct.tile([128, F], f16, tag="t2")
                t3 = accp.tile([128, F], f16, tag="t3")
                u0 = accp.tile([128, F], f16, tag="u0")
                u1 = accp.tile([128, F], f16, tag="u1")
                ot = accb.tile([128, F], f16, tag="ot")
                nc.scalar.activation(t2[:], it[:, 2 * F:3 * F], act_copy,
                                     scale=w[2])
                nc.vector.tensor_scalar(t0[:], it[:, 0:F], w[0], bias128,
                                        op0=alu.mult, op1=alu.add)
                nc.vector.tensor_scalar(t1[:], it[:, F:2 * F], w[1], None,
                                        op0=alu.mult)
                nc.vector.tensor_scalar(t3[:], it[:, 3 * F:4 * F], w[3],
                                        None, op0=alu.mult)
                nc.vector.tensor_tensor(u0[:], t0[:], t1[:], op=alu.add)
                nc.vector.tensor_tensor(u1[:], t2[:], t3[:], op=alu.add)
                nc.vector.tensor_tensor(ot[:], u0[:], u1[:], op=alu.add)
                if ti < len(planes) - 2:
                    nc.scalar.dma_start(out=out[:, ooff:ooff + F],
                                        in_=ot[:])
                else:
                    nc.sync.dma_start(out=out[:, ooff:ooff + F],
                                      in_=ot[:])
                ooff += F

            # stats ride the tail of the SP ring (host only needs them
            # after the run, for the eigenvector sign check)
            nc.sync.dma_start(out=stats[:], in_=t1b[:])
    _split_sync_waits(nc)
    return nc


def _get_programs():
    global _programs
    if _programs is None:
        _programs = (_build_pass1(), _build_pass2())
    return _programs


def _get_fused():
    global _fused_program
    if _fused_program is None:
        _fused_program = _build_fused()
    return _fused_program


def _host_fold(stats8):
    """stats8: [B//SPC, 8, 130] f32 device-folded stats -> per-sample
    mu [B,4], sigma [B,4], comp [B,4] (reference-sign top eigenvector).

    Row 4s+m of a core's [8, 130] block holds M_s[4*(col//4)+m, col] for
    col<128 (exact copies of the block-diagonal entries) and colsum_s[m]
    at col 128.  Downstream matches the reference exactly: gram from
    (S - N mu mu^T) / (sigma sigma^T), comp = eigh(gram f32) top
    eigenvector on CPU jax.
    """
    st = stats8.astype(np.float64).reshape(B, 4, BST1)   # [b, m, col]
    t1 = st[:, :, :128].reshape(B, 4, 32, 4)             # [b, k, g, l]
    S = t1.sum(axis=2)                                   # [B, 4, 4]
    colsum = st[:, :, 128]                               # [B, 4]

    mu = colsum / NROWS
    e2 = np.einsum("bkk->bk", S) / NROWS
    var = np.maximum(e2 - mu * mu, 0.0)
    sigma = np.sqrt(var)
    denom = sigma[:, :, None] * sigma[:, None, :]
    gram = (S - NROWS * mu[:, :, None] * mu[:, None, :])
    with np.errstate(divide="ignore", invalid="ignore"):
        gram = np.where(denom > 0, gram / np.where(denom > 0, denom, 1.0), 0.0)

    # eigh with the same implementation/backend the reference uses (CPU jax)
    import jax
    import jax.numpy as jnp
    with jax.default_device(jax.devices("cpu")[0]):
        V = np.asarray(jnp.linalg.eigh(jnp.asarray(gram, jnp.float32))[1])
    comp = V[:, :, -1].astype(np.float64)                # top eigenvector
    return mu, sigma, comp


def _host_comp(stats8):
    return _host_fold(stats8)[2]


def _host_middle(stats8):
    """stats8 -> w [B, 4] f64, bias [B] f64 for the host-prescale path."""
    mu, sigma, comp = _host_fold(stats8)
    with np.errstate(divide="ignore", invalid="ignore"):
        w = np.where(sigma > 0, comp / np.where(sigma > 0, sigma, 1.0), 0.0)
    bias = -(mu * w).sum(axis=1)
    return w, bias


def _layouts(x16):
    """Build the gram-pass and plane-pass device layouts from fp16 x."""
    xp = np.zeros((B, 128, NBLK1, BST1), np.float16)
    xp[..., :128] = x16.reshape(B, NBLK1, 128, 128).transpose(0, 2, 1, 3)
    xp[..., 128] = 1.0
    xp = xp.reshape(B, 128, NBLK1 * BST1)

    xpl = x16.reshape(B, HO, 2, WO, 2, C // 4, 4).transpose(
        0, 1, 3, 6, 2, 4, 5)
    xpl = np.ascontiguousarray(xpl).reshape(B, 64, 49, 4, C)
    segs = []
    oo0 = 0
    for oo in FP2_TILES:
        seg = xpl[:, :, oo0:oo0 + oo].transpose(0, 1, 3, 2, 4)
        segs.append(seg.reshape(B, 64, 4 * oo * C))
        oo0 += oo
    x2h = np.concatenate(segs, axis=2)             # [B, 64, 49*4*C]
    return xp, x2h


def _layout_p2_scaled(x16, w, bias):
    """k-plane pass-2 layout with w_k folded into each plane and the bias
    folded into plane 0 (host knows w after pass 1; the scale rides the
    layout pass that exists anyway).  Returns x2h [B, 64, 49*4*C] fp16."""
    xpl = x16.reshape(B, HO, 2, WO, 2, C // 4, 4).transpose(
        0, 1, 3, 6, 2, 4, 5)
    xpl = np.ascontiguousarray(xpl).reshape(B, 3136, 4, C)
    xs = xpl.astype(np.float32)
    xs *= w.astype(np.float32)[:, None, :, None]
    xs[:, :, 0, :] += bias.astype(np.float32)[:, None, None]
    xpl = xs.astype(np.float16).reshape(B, 64, 49, 4, C)
    segs = []
    oo0 = 0
    for oo in P2_TILES:
        seg = xpl[:, :, oo0:oo0 + oo].transpose(0, 1, 3, 2, 4)
        segs.append(seg.reshape(B, 64, 4 * oo * C))
        oo0 += oo
    return np.concatenate(segs, axis=2)             # [B, 64, 49*4*C]


def _kernel_fused(x16):
    from concourse.bass_utils import run_bass_kernel_spmd

    ncf = _get_fused()
    core_ids = list(range(N_CORES))
    xp, x2h = _layouts(x16)
    cst = _host_consts()
    ins = []
    for c in range(N_CORES):
        pair = x2h[c * SPC:(c + 1) * SPC]
        ins.append({
            "xg": xp[c * SPC:(c + 1) * SPC],
            "xp": pair.reshape(128, 49 * 4 * C),
            "cst": cst,
        })
    kw = dict(trace=True, tmpdir=TRACE_DIRS.get("pass1")) if TRACE else {}
    r = run_bass_kernel_spmd(ncf, ins, core_ids, **kw)
    if TRACE:
        LAST_PROFILE["pass1_ns"] = r.exec_time_ns
        LAST_PROFILE["pass2_ns"] = 0

    # Sign fix: the device's power iteration returns comp * sign(comp.r8)
    # (even power of a positive-top-eig matrix applied to the fixed probe
    # r8), while the reference's eigh sign is whatever LAPACK produced.
    # comp_host from the device-folded stats tells us both.
    stats8 = np.stack([r.results[c]["stats"] for c in range(N_CORES)])
    comp = _host_comp(stats8)                     # [B, 4] reference-sign
    rp = np.array(R_PROBE, np.float64)
    flip = (comp @ rp) < 0                        # device sign != host sign

    outs = []
    for c in range(N_CORES):
        o = r.results[c]["out"].astype(np.float32).reshape(SPC, HO, WO, C)
        for s in range(SPC):
            if flip[c * SPC + s]:
                o[s] = -o[s]
        outs.append(o)
    return np.ascontiguousarray(np.concatenate(outs))


def kernel(x):
    from concourse.bass_utils import run_bass_kernel_spmd

    x = np.asarray(x)
    assert x.shape == (B, H, W, C), x.shape
    x16 = np.ascontiguousarray(x, dtype=np.float16)
    if FUSED:
        return _kernel_fused(x16)
    nc1, nc2 = _get_programs()
    core_ids = list(range(N_CORES))

    # pass-1 input: 128-row x 128-col blocks padded to 130 cols with a
    # ones column at 128 ([128 partitions, blocks]); row r = pix*2 + half
    xp = np.zeros((B, 128, NBLK1, BST1), np.float16)
    xp[..., :128] = x16.reshape(B, NBLK1, 128, 128).transpose(0, 2, 1, 3)
    xp[..., 128] = 1.0
    xp = xp.reshape(B, 128, NBLK1 * BST1)
    cst1 = _p1_consts()
    in1 = [{"x": xp[c * SPC:(c + 1) * SPC], "cst": cst1}
           for c in range(N_CORES)]
    kw1 = dict(trace=True, tmpdir=TRACE_DIRS.get("pass1")) if TRACE else {}
    r1 = run_bass_kernel_spmd(nc1, in1, core_ids, **kw1)
    if TRACE:
        LAST_PROFILE["pass1_ns"] = r1.exec_time_ns
    stats8 = np.stack([r1.results[c]["stats"] for c in range(N_CORES)])

    w, bias = _host_middle(stats8)
    x2h = _layout_p2_scaled(x16, w, bias)
    in2 = []
    for c in range(N_CORES):
        pair = x2h[c * SPC:(c + 1) * SPC]          # [2, 64, 49*4*C]
        in2.append({"x": pair.reshape(128, 49 * 4 * C)})
    kw2 = dict(trace=True, tmpdir=TRACE_DIRS.get("pass2")) if TRACE else {}
    r2 = run_bass_kernel_spmd(nc2, in2, core_ids, **kw2)
    if TRACE:
        LAST_PROFILE["pass2_ns"] = r2.exec_time_ns

    # gather: out[s*64+p64, oo*C+c'], outpix = p64*49+oo -> [B, HO, WO, C]
    outs = [r2.results[c]["out"].astype(np.float32).reshape(SPC, HO, WO, C)
            for c in range(N_CORES)]
    return np.ascontiguousarray(np.concatenate(outs))



# revision 34
# speedup vs baseline: 2.7991x; 1.1178x over previous
"""BPCA pooling layer on 8 Trainium2 NeuronCores (Bass/Tile).

Math: per sample, the reference's `data = patches.reshape(-1, 4)` groups 4
consecutive channels (C=256 is divisible by 4), so `data` is exactly the
sample's contiguous buffer viewed as [N, 4] with N = H*W*C/4.  The layer is:

  1. per-column mean/std over N rows, dn = (data-mean)/std
  2. gram = dn^T dn (4x4), comp = top eigenvector (jnp.linalg.eigh)
  3. out = (dn @ comp) reshaped to [H/2, W/2, C] with channel permutation
     c' = (2*di+dj)*64 + (c//4)

Device plan (2 samples per core, pure data parallel).  Both passes are
DMA-bound, so all device I/O except the tiny stats tensor is fp16 --
quantizing x to fp16 perturbs the final output by ~3e-4 rel (measured
against the fixed seed), 60x under the 2e-2 gate, and halves traffic:

  pass 1: PE computes the 128x128 half-channel second-moment matrix
          M[j,j'] = sum_{pix,G} x[pix,128G+j]*x[pix,128G+j'] plus column
          sums (ones column), accumulated in fp32 PSUM, from fp16 inputs.
          128-col blocks (one matmul per 128-row block, N=130 moving)
          instead of 256-col halves: same LDWEIGHTS count but half the
          moving columns, so the PE stream (~81ns/MM warm) stays under
          the DMA stream.
  host:   fold M into the 4x4 gram (S_kl = sum_g M[4g+k,4g+l]), compute
          mean/std/gram in f64, eigh on CPU jax (same implementation the
          reference uses), derive w_k = comp_k/std_k and
          bias = -sum_k mean_k*comp_k/std_k.
  pass 2: out = sum of four host-prescaled k-planes (the host builds the
          k-plane layout after pass 1, when it already has w/bias, so the
          per-plane scale w_k and the bias ride that existing layout
          pass) -- three tensor_tensor adds on DVE (2x mode for packed
          fp16), far under the DMA stream.  NOT a scalar_tensor_tensor
          chain: STT has no accelerated DVE uops and runs 1 elem/cyc.
          Output channel permutation is folded into the host layout.

All bulk loads go through the single SP DMA queue: one queue aggregates
~350 GB/s across the 16 DMA engines, while splitting across two queues
measured LOWER total (the engines, not the queue, are the resource);
the ACT queue only carries small stores (it gets poor engine service).
Graduated tile ladders (small head, small tail) plus bufs=8 prefetch
keep the whole-tile DMA-completion semaphores off the critical path.
"""

import numpy as np

# ---------------------------------------------------------------------------
# Problem constants (hardcoded per spec)
# ---------------------------------------------------------------------------
B, H, W, C = 16, 112, 112, 256
N_CORES = 8
SPC = B // N_CORES          # samples per core = 2
PIX = H * W                 # 12544 pixels per sample
NBLK1 = PIX * C // (128 * 128)  # 196 row-blocks of 128 per sample
BST1 = 130                  # per-block SBUF cols: 128 data + 1 ones + 1 pad
P1_TILES = [32, 32, 32, 32, 32, 24, 12]  # (sum=196)
# uniform 32-block tiles keep 8.3KB per-partition DMA segments through the
# stream (the old graduated head ran 0.5-5KB segments and measurably
# dropped the stream rate); the shrinking tail bounds the PE chase after
# the last tile lands
NROWS = PIX * C // 4        # 802816 rows of the [N, 4] data matrix
HO, WO = H // 2, W // 2     # 56 x 56 output

_programs = None
_fused_program = None
LAST_PROFILE = {}
TRACE = False
TRACE_DIRS = {}
FUSED = True                # single NEFF: pays the ~9us queue-startup head
                            # and the ~2.3us end fence once instead of twice
NSQ = 9                     # matrix squarings (power 512; sim err 8e-5)
P2_TILES = [2, 4, 8, 12, 12, 9, 2]
FP2_TILES = [16, 16, 12, 5]  # fused plane chunks: few big DVE ops (the
                             # ~0.5us fixed cost per DVE op dominates at
                             # small tile sizes), small tail for the chase
CONST_COLS = 332
P1_CONST_COLS = 146


# ---------------------------------------------------------------------------
# TileContext with a walrus-compatible tail drain
# ---------------------------------------------------------------------------
def _make_tile_context(nc):
    from concourse.tile import TileContext
    return TileContext(nc)


def _split_sync_waits(nc):
    """walrus (CoreV2/V3 codegen) rejects instructions carrying more than 2
    sync commands (waits + updates combined); Tile freely emits e.g. 2 waits
    + 1 update.  Hoist excess waits onto same-engine NOPs inserted directly
    before the offending instruction -- same engine means the same program-
    order point, so semantics are unchanged."""
    import concourse.mybir as mybir

    def mint_nop(engine):
        inner = nc.engines[engine].nop().ins
        for blk in nc.m.functions[0].blocks:
            il = blk.instructions
            for k in range(len(il) - 1, -1, -1):
                if il[k] is inner:
                    il.pop(k)
                    return inner
        raise RuntimeError("minted nop not found in any block")

    for fn in nc.m.functions:
        for blk in fn.blocks:
            il = blk.instructions
            i = 0
            while i < len(il):
                inst = il[i]
                si = inst.sync_info
                waits = list(si.on_wait) if si and si.on_wait else []
                upds = list(si.on_update) if si and si.on_update else []
                # observed walrus limits: at most 1 wait per instruction
                # (1 wait + 1 update compiles; 2 waits anywhere does not)
                if len(waits) > 1:
                    extra, keep = waits[:-1], waits[-1:]
                    for wchunk in extra:
                        nop = mint_nop(inst.engine)
                        nop.sync_info = mybir.SyncInfo(
                            on_wait=[wchunk], on_update=[])
                        il.insert(i, nop)
                        i += 1
                    inst.sync_info = mybir.SyncInfo(
                        on_wait=keep, on_update=upds)
                i += 1


def _p1_consts():
    """[128, 146] f32: cols 0:130 block-diag fold mask (col 128 = ones for
    the chansum column, col 129 = 0 to kill the pad col), 130:138 sample-0
    row selector (p%4==m in col 130+m), 138:146 sample-1 selector (col
    142+m)."""
    ct = np.zeros((128, P1_CONST_COLS), np.float32)
    p = np.arange(128)
    q = np.arange(128)
    ct[:, 0:128] = (p[:, None] // 4 == q[None, :] // 4).astype(np.float32)
    ct[:, 128] = 1.0
    for m in range(4):
        ct[:, 130 + m] = (p % 4 == m)
        ct[:, 142 + m] = (p % 4 == m)
    return ct


def _build_pass1():
    import concourse.bass as bass
    import concourse.mybir as mybir

    f32 = mybir.dt.float32
    f16 = mybir.dt.float16
    alu = mybir.AluOpType

    nc = bass.Bass("TRN2", target_bir_lowering=False, debug=False,
                   num_devices=N_CORES)
    # The host pre-interleaves a ones column per block (col 128 of each
    # 130-wide block) so one DMA loads data + ones and no on-device memset
    # is needed.
    x = nc.dram_tensor("x", [SPC, 128, NBLK1 * BST1], f16,
                       kind="ExternalInput").ap()
    cst = nc.dram_tensor("cst", [128, P1_CONST_COLS], f32,
                         kind="ExternalInput").ap()
    # folded stats: rows 4s+m hold M_s[4*(col//4)+m, col] for col<128 (an
    # exact selector-matmul copy of the block-diag entries the host fold
    # uses) and colsum_s[m] in col 128.  4KB store instead of the old
    # 2x[128,130] (255KB) whose 288-packet drain on the scalar queue was
    # ~5us of critical-path tail.
    stats = nc.dram_tensor("stats", [8, BST1], f32,
                           kind="ExternalOutput").ap()

    with _make_tile_context(nc) as tc:
        with (
            tc.tile_pool(name="cstp", bufs=1) as cstp,
            tc.tile_pool(name="inp", bufs=8) as inp,
            tc.tile_pool(name="psum", bufs=2, space="PSUM") as psum,
            tc.tile_pool(name="pst", bufs=1, space="PSUM") as pst,
            tc.tile_pool(name="sml", bufs=1) as sml,
        ):
            # const load on the ACT queue: the load queue stays clean and
            # the transfer (75KB) completes long before the first fold use
            ct = cstp.tile([128, P1_CONST_COLS], f32, tag="cst")
            nc.scalar.dma_start(out=ct[:], in_=cst[:])
            mask130 = ct[:, 0:130]
            lsels = [ct[:, 130:138], ct[:, 138:146]]

            bms = []
            for s in range(SPC):
                ps = psum.tile([128, BST1], f32, tag="ps")
                b0 = 0
                for nb in P1_TILES:
                    t = inp.tile([128, nb * BST1], f16, tag="in")
                    t3 = t[:].rearrange("p (j b) -> p j b", b=BST1)
                    nc.sync.dma_start(
                        out=t[:],
                        in_=x[s, :, b0 * BST1:(b0 + nb) * BST1])
                    for j in range(nb):
                        first = b0 + j == 0
                        last = b0 + j == NBLK1 - 1
                        nc.tensor.matmul(ps[:, 0:BST1],
                                         t3[:, j:j + 1, 0:128],
                                         t3[:, j:j + 1, 0:BST1],
                                         start=first, stop=last,
                                         skip_group_check=True)
                    b0 += nb
                # block-diag mask applied straight from PSUM (sample 0's
                # runs mid-stream on the idle DVE)
                bm = sml.tile([128, BST1], f32, tag=f"bm{s}")
                nc.vector.tensor_tensor(bm[:], ps[:, 0:BST1], mask130,
                                        op=alu.mult)
                bms.append(bm)
            # both selector matmuls at the end so the PE gram streams are
            # never interrupted by a wait on the DVE mask-mult
            t1p = pst.tile([8, BST1], f32, tag="t1p")
            for s in range(SPC):
                nc.tensor.matmul(t1p[:], lsels[s], bms[s][:],
                                 start=(s == 0), stop=(s == SPC - 1),
                                 skip_group_check=True)
            t1b = sml.tile([8, BST1], f32, tag="t1b")
            nc.vector.tensor_copy(out=t1b[:], in_=t1p[:])
            # the load queue is idle once the last load retires, so this
            # drains immediately (the ACT queue adds ~3us of service lag)
            nc.sync.dma_start(out=stats[:], in_=t1b[:])
    _split_sync_waits(nc)
    return nc


def _build_pass2():
    import concourse.bass as bass
    import concourse.mybir as mybir

    f16 = mybir.dt.float16
    alu = mybir.AluOpType
    OO = 49  # output pixels per partition (3136 = 64 partitions x 49)

    nc = bass.Bass("TRN2", target_bir_lowering=False, debug=False,
                   num_devices=N_CORES)
    # Partition p = (s_local*64 + p64): both samples fill 128 partitions so
    # every DMA is a fully-contiguous 128-partition transfer.  The host
    # builds the k-plane layout AFTER pass 1 (it already has w/bias then),
    # so the per-plane scale w_k and the bias (folded into plane 0) ride
    # the existing host layout pass; the device sums the four planes with
    # three tensor_tensor adds (2x DVE mode for packed fp16).
    x = nc.dram_tensor("x", [128, OO * 4 * C], f16,
                       kind="ExternalInput").ap()
    out = nc.dram_tensor("out", [128, OO * C], f16,
                         kind="ExternalOutput").ap()

    with _make_tile_context(nc) as tc:
        with (
            tc.tile_pool(name="inp", bufs=4) as inp,
            tc.tile_pool(name="acc", bufs=2) as accp,
            tc.tile_pool(name="otp", bufs=4) as otp,
        ):
            off = 0
            ooff = 0
            for ti, oo in enumerate(P2_TILES):
                F = oo * C
                it = inp.tile([128, 4 * F], f16, tag="it")
                nc.sync.dma_start(out=it[:], in_=x[:, off:off + 4 * F])
                u0 = accp.tile([128, F], f16, tag="u0")
                u1 = accp.tile([128, F], f16, tag="u1")
                # deep ot pool: DVE never waits on ACT-queue store service
                ot = otp.tile([128, F], f16, tag="ot")
                nc.vector.tensor_tensor(
                    u0[:], it[:, 0:F], it[:, F:2 * F], op=alu.add)
                nc.vector.tensor_tensor(
                    u1[:], it[:, 2 * F:3 * F], it[:, 3 * F:4 * F],
                    op=alu.add)
                nc.vector.tensor_tensor(
                    ot[:], u0[:], u1[:], op=alu.add)
                if ti < len(P2_TILES) - 2:
                    # mid-stream stores on the ACT queue: the load queue
                    # stays free to prefetch
                    nc.scalar.dma_start(
                        out=out[:, ooff:ooff + F], in_=ot[:])
                else:
                    # the last stores ride the load queue, which is idle
                    # once the final load descriptor retires and drains
                    # far faster than the ACT queue
                    nc.sync.dma_start(
                        out=out[:, ooff:ooff + F], in_=ot[:])
                off += 4 * F
                ooff += F
    _split_sync_waits(nc)
    return nc


R_PROBE = (0.5393, -0.2117, 0.8313, 0.1078)  # fixed eig probe (per k)
ESHIFT = 0.02                                # a8 = gram/(ESHIFT*N) - 49*I


def _host_consts():
    """Constant tensor for the fused kernel's on-device fold/eigensolve."""
    ct = np.zeros((128, CONST_COLS), np.float32)
    p = np.arange(128)
    q = np.arange(128)
    p8 = np.arange(8)
    # 0:130 block-diag mask for M -> per-group fold (col 128 = chansums)
    ct[:, 0:128] = (p[:, None] // 4 == q[None, :] // 4).astype(np.float32)
    ct[:, 128] = 1.0
    # 130:138 / 138:146 per-sample fold selectors (PSUM-accumulated);
    # rows 0:8 of 130:134 double as lseld (p%4==l diag selector)
    for m in range(4):
        ct[:, 130 + m] = (p % 4 == m)
        ct[:, 142 + m] = (p % 4 == m)
    # 146:154 I8, 154:162 blockones8
    ct[0:8, 146:154] = np.eye(8, dtype=np.float32)
    ct[0:8, 154:162] = (p8[:, None] // 4 == p8[None, :] // 4)
    # 162 r8 (fixed probe vector, repeated per sample)
    ct[0:8, 162] = np.array(R_PROBE, np.float32)[p8 % 4]
    # 163:171 / 171:179 half-diagonal projectors (block-diag embed of S)
    ct[0:8, 163:171] = np.diag((p8 < 4).astype(np.float32))
    ct[0:8, 171:179] = np.diag((p8 >= 4).astype(np.float32))
    # 179:187 -49*I8; 187:195 m8/(ESHIFT*N); 195:203 -m8/ESHIFT
    ct[0:8, 179:187] = -49.0 * np.eye(8, dtype=np.float32)
    ct[0:8, 187:195] = ct[0:8, 154:162] / (ESHIFT * NROWS)
    ct[0:8, 195:203] = ct[0:8, 154:162] * (-1.0 / ESHIFT)
    # 203: -(p%4==0) (bias extraction, sign folded in)
    ct[0:8, 203] = -(p8 % 4 == 0).astype(np.float32)
    # 204:332 SEL2 [8,128]: (p//4 == q//64) broadcasts per-sample w/bias
    # rows to that sample's 64 projection partitions in one matmul
    ct[0:8, 204:332] = (p8[:, None] // 4 == q[None, :] // 64)
    return ct


def _build_fused():
    import concourse.bass as bass
    import concourse.mybir as mybir

    f32 = mybir.dt.float32
    f16 = mybir.dt.float16
    alu = mybir.AluOpType
    actf = mybir.ActivationFunctionType
    act_copy = actf.Copy
    ax = mybir.AxisListType
    OO = 49

    nc = bass.Bass("TRN2", target_bir_lowering=False, debug=False,
                   num_devices=N_CORES)
    xg = nc.dram_tensor("xg", [SPC, 128, NBLK1 * BST1], f16,
                        kind="ExternalInput").ap()
    xp = nc.dram_tensor("xp", [128, OO * 4 * C], f16,
                        kind="ExternalInput").ap()
    cst = nc.dram_tensor("cst", [128, CONST_COLS], f32,
                         kind="ExternalInput").ap()
    out = nc.dram_tensor("out", [128, OO * C], f16,
                         kind="ExternalOutput").ap()
    stats = nc.dram_tensor("stats", [8, BST1], f32,
                           kind="ExternalOutput").ap()

    with _make_tile_context(nc) as tc:
        with (
            tc.tile_pool(name="cstp", bufs=1) as cstp,
            tc.tile_pool(name="inp", bufs=8) as inp,
            tc.tile_pool(name="pin", bufs=1) as pin,
            tc.tile_pool(name="psum", bufs=1, space="PSUM") as psum,
            tc.tile_pool(name="pst", bufs=1, space="PSUM") as pst,
            tc.tile_pool(name="sml", bufs=1) as sml,
            tc.tile_pool(name="eig", bufs=3) as eig,
        ):
            # const load on the ACT queue: ready long before first use
            ct = cstp.tile([128, CONST_COLS], f32, tag="cst")
            nc.scalar.dma_start(out=ct[:], in_=cst[:])
            mask130 = ct[:, 0:130]
            lsel0 = ct[:, 130:138]
            lsel1 = ct[:, 138:146]
            lseld = ct[0:8, 130:134]     # (p%4==l) diag selector rows 0-7
            i8 = ct[0:8, 146:154]
            m8 = ct[0:8, 154:162]
            r8 = ct[0:8, 162:163]
            p0d = ct[0:8, 163:171]
            p1d = ct[0:8, 171:179]
            i8m49 = ct[0:8, 179:187]
            m8s = ct[0:8, 187:195]
            m8n50 = ct[0:8, 195:203]
            maskp0n = ct[0:8, 203:204]
            sel2 = ct[0:8, 204:332]

            # ---- phase 1: gram matmuls over the site-block stream -------
            # (all bulk loads on the single SP queue: one queue aggregates
            # the 16 DMA engines and orders the gram stream ahead of the
            # plane stream with no gate descriptors)
            pss = []
            for s in range(SPC):
                ps = psum.tile([128, BST1], f32, tag=f"ps{s}")
                b0 = 0
                for nb in P1_TILES:
                    t = inp.tile([128, nb * BST1], f16, tag="in")
                    t3 = t[:].rearrange("p (j b) -> p j b", b=BST1)
                    nc.sync.dma_start(
                        out=t[:], in_=xg[s, :, b0 * BST1:(b0 + nb) * BST1])
                    for j in range(nb):
                        nc.tensor.matmul(ps[:, 0:BST1],
                                         t3[:, j:j + 1, 0:128],
                                         t3[:, j:j + 1, 0:BST1],
                                         start=(b0 + j == 0),
                                         stop=(b0 + j == NBLK1 - 1),
                                         skip_group_check=True)
                    b0 += nb
                pss.append(ps)

            # ---- queue all plane loads (strictly after the gram loads) --
            planes = []
            off = 0
            for oo in FP2_TILES:
                F = oo * C
                it = pin.tile([128, 4 * F], f16, tag=f"it{len(planes)}")
                nc.sync.dma_start(out=it[:], in_=xp[:, off:off + 4 * F])
                planes.append((it, F))
                off += 4 * F

            # ---- fold: masked PSUM reads + selector matmuls ------------
            bms = []
            for s in range(SPC):
                bm = sml.tile([128, BST1], f32, tag=f"bm{s}")
                nc.vector.tensor_tensor(bm[:], pss[s][:, 0:BST1], mask130,
                                        op=alu.mult)
                bms.append(bm)
            t1p = pst.tile([8, BST1], f32, tag="t1p")
            for s in range(SPC):
                nc.tensor.matmul(t1p[:], lsel0 if s == 0 else lsel1,
                                 bms[s][:], start=(s == 0),
                                 stop=(s == SPC - 1), skip_group_check=True)
            t1b = sml.tile([8, BST1], f32, tag="t1b")
            nc.vector.tensor_copy(out=t1b[:], in_=t1p[:])
            s8 = sml.tile([8, 4], f32, tag="s8")
            nc.vector.tensor_reduce(
                s8[:], t1b[:, 0:128].rearrange("p (g l) -> p l g", l=4),
                axis=ax.X, op=alu.add)
            mu8 = sml.tile([8, 1], f32, tag="mu8")
            nc.vector.tensor_scalar(mu8[:], t1b[:, 128:129],
                                    float(1.0 / NROWS), None, op0=alu.mult)
            tmp84 = sml.tile([8, 4], f32, tag="tmp84")
            nc.vector.tensor_tensor(tmp84[:], s8[:], lseld, op=alu.mult)
            e2 = sml.tile([8, 1], f32, tag="e2")
            nc.vector.tensor_reduce(e2[:], tmp84[:], axis=ax.X, op=alu.add)
            m2 = sml.tile([8, 1], f32, tag="m2")
            nc.vector.tensor_tensor(m2[:], mu8[:], mu8[:], op=alu.mult)
            var8 = sml.tile([8, 1], f32, tag="var8")
            nc.vector.scalar_tensor_tensor(var8[:], e2[:],
                                           float(1.0 / NROWS), m2[:],
                                           op0=alu.mult, op1=alu.subtract)
            std8 = sml.tile([8, 1], f32, tag="std8")
            nc.scalar.activation(std8[:], var8[:], actf.Sqrt)
            rstd8 = sml.tile([8, 1], f32, tag="rstd8")
            nc.vector.reciprocal(rstd8[:], std8[:])

            # block-diag embed of the two 4x4 S matrices via masked PE
            # matmuls (DVE cannot address partition ranges off base 0)
            sembp = pst.tile([8, 8], f32, tag="o8")
            nc.tensor.matmul(sembp[:, 0:4], p0d, s8[:],
                             start=True, stop=True)
            nc.tensor.matmul(sembp[:, 4:8], p1d, s8[:],
                             start=True, stop=True)
            semb = sml.tile([8, 8], f32, tag="semb")
            nc.vector.tensor_copy(out=semb[:], in_=sembp[:])

            # D S D via two diag-matmuls, D = diag(1/sigma)
            dstd = sml.tile([8, 8], f32, tag="dstd")
            nc.vector.tensor_scalar(dstd[:], i8, rstd8[:], None,
                                    op0=alu.mult)
            s1p = pst.tile([8, 8], f32, tag="o8")
            nc.tensor.matmul(s1p[:], semb[:], dstd[:], start=True,
                             stop=True)
            s1c = sml.tile([8, 8], f32, tag="s1c")
            nc.vector.tensor_copy(out=s1c[:], in_=s1p[:])
            sddp = pst.tile([8, 8], f32, tag="o8")
            nc.tensor.matmul(sddp[:], dstd[:], s1c[:], start=True,
                             stop=True)
            sdd = sml.tile([8, 8], f32, tag="sdd")
            nc.vector.tensor_copy(out=sdd[:], in_=sddp[:])

            # nu nu^T outer product (nu = mu/sigma) via PE transpose
            nu = sml.tile([8, 1], f32, tag="nu")
            nc.vector.tensor_tensor(nu[:], mu8[:], rstd8[:], op=alu.mult)
            nutp = pst.tile([1, 8], f32, tag="tr")
            nc.tensor.transpose(nutp[:], nu[:], i8)
            nut = sml.tile([1, 8], f32, tag="nut")
            nc.vector.tensor_copy(out=nut[:], in_=nutp[:])
            onnp = pst.tile([8, 8], f32, tag="o8")
            nc.tensor.matmul(onnp[:], nut[:], nut[:], start=True, stop=True)
            onn = sml.tile([8, 8], f32, tag="onn")
            nc.vector.tensor_copy(out=onn[:], in_=onnp[:])

            # a8 = gram/(ESHIFT*N) - 49*I  (eigs land at 1 + 50*delta so a
            # single mid-chain fro normalization suffices)
            g8i = sml.tile([8, 8], f32, tag="g8i")
            nc.vector.scalar_tensor_tensor(g8i[:], sdd[:],
                                           float(1.0 / (ESHIFT * NROWS)),
                                           i8m49, op0=alu.mult, op1=alu.add)
            onnm = sml.tile([8, 8], f32, tag="onnm")
            nc.vector.tensor_tensor(onnm[:], onn[:], m8n50, op=alu.mult)
            a8 = eig.tile([8, 8], f32, tag="a8")
            nc.vector.tensor_tensor(a8[:], g8i[:], onnm[:], op=alu.add)

            # ---- eigensolve: repeated squaring, one per-block fro norm --
            for t in range(NSQ):
                if t == 5:
                    sq = eig.tile([8, 8], f32, tag="sq")
                    nc.vector.tensor_tensor(sq[:], a8[:], a8[:],
                                            op=alu.mult)
                    rs = eig.tile([8, 1], f32, tag="rs")
                    nc.vector.tensor_reduce(rs[:], sq[:], axis=ax.X,
                                            op=alu.add)
                    fbp = pst.tile([8, 1], f32, tag="v1")
                    nc.tensor.matmul(fbp[:], m8, rs[:], start=True,
                                     stop=True)
                    rfb = eig.tile([8, 1], f32, tag="fb")
                    nc.vector.reciprocal(rfb[:], fbp[:])
                    rfrt = eig.tile([8, 1], f32, tag="rfrt")
                    nc.scalar.activation(rfrt[:], rfb[:], actf.Sqrt)
                    an = eig.tile([8, 8], f32, tag="a8")
                    nc.vector.tensor_scalar(an[:], a8[:], rfrt[:], None,
                                            op0=alu.mult)
                    a8 = an
                a2p = pst.tile([8, 8], f32, tag="o8")
                nc.tensor.matmul(a2p[:], a8[:], a8[:], start=True,
                                 stop=True)
                a8 = eig.tile([8, 8], f32, tag="a8")
                nc.vector.tensor_copy(out=a8[:], in_=a2p[:])

            # ---- top eigenvector, w, bias ------------------------------
            v8p = pst.tile([8, 1], f32, tag="v1")
            nc.tensor.matmul(v8p[:], a8[:], r8, start=True, stop=True)
            v8 = sml.tile([8, 1], f32, tag="v8")
            nc.vector.tensor_copy(out=v8[:], in_=v8p[:])
            vsq = sml.tile([8, 1], f32, tag="vsq")
            nc.vector.tensor_tensor(vsq[:], v8[:], v8[:], op=alu.mult)
            nbp = pst.tile([8, 1], f32, tag="v1")
            nc.tensor.matmul(nbp[:], m8, vsq[:], start=True, stop=True)
            rnb = sml.tile([8, 1], f32, tag="rnb")
            nc.vector.reciprocal(rnb[:], nbp[:])
            rnrt = sml.tile([8, 1], f32, tag="rnrt")
            nc.scalar.activation(rnrt[:], rnb[:], actf.Sqrt)
            w8 = sml.tile([8, 1], f32, tag="w8")
            nc.vector.scalar_tensor_tensor(w8[:], v8[:], rnrt[:],
                                           rstd8[:], op0=alu.mult,
                                           op1=alu.mult)
            prod = sml.tile([8, 1], f32, tag="prod")
            nc.vector.tensor_tensor(prod[:], mu8[:], w8[:], op=alu.mult)
            pbp = pst.tile([8, 1], f32, tag="v1")
            nc.tensor.matmul(pbp[:], m8, prod[:], start=True, stop=True)
            pb = sml.tile([8, 1], f32, tag="pb")
            nc.vector.tensor_copy(out=pb[:], in_=pbp[:])

            # ---- broadcast w/bias to 128 partitions in ONE matmul ------
            # wb5[p,k] = w8[p]*(p%4==k), wb5[p,4] = -pb[p]*(p%4==0);
            # SEL2 sums each sample block onto its 64 partitions.
            wb5 = sml.tile([8, 5], f32, tag="wb5")
            nc.vector.tensor_scalar(wb5[:, 0:4], lseld, w8[:], None,
                                    op0=alu.mult)
            nc.vector.tensor_scalar(wb5[:, 4:5], maskp0n, pb[:], None,
                                    op0=alu.mult)
            wbp = pst.tile([128, 5], f32, tag="wbp")
            nc.tensor.matmul(wbp[:], sel2, wb5[:], start=True, stop=True)
            wball = sml.tile([128, 5], f32, tag="wball")
            nc.vector.tensor_copy(out=wball[:], in_=wbp[:])
            wl4 = wball[:, 0:4]
            bias128 = wball[:, 4:5]

            # ---- projection (pass-2 compute) ---------------------------
            # All in-place inside the plane tiles (no acc pools): DVE does
            # TS + two STT (fused mult+add) + one TT per chunk; ACT takes
            # the k=3 scale.  Stores: first chunk on the ACT queue (its
            # issue placed AFTER all ACT scales so it never interlocks the
            # projection), the rest on the then-idle SP queue.
            w = [wl4[:, k:k + 1] for k in range(4)]
            acc_spans = []
            for ti, (it, F) in enumerate(planes):
                p0 = it[:, 0:F]
                p1 = it[:, F:2 * F]
                p2 = it[:, 2 * F:3 * F]
                p3 = it[:, 3 * F:4 * F]
                nc.scalar.activation(p3, p3, act_copy, scale=w[3])
                nc.vector.tensor_scalar(p0, p0, w[0], bias128,
                                        op0=alu.mult, op1=alu.add)
                nc.vector.scalar_tensor_tensor(p0, p1, w[1], p0,
                                               op0=alu.mult, op1=alu.add)
                nc.vector.scalar_tensor_tensor(p0, p2, w[2], p0,
                                               op0=alu.mult, op1=alu.add)
                nc.vector.tensor_tensor(p0, p0, p3, op=alu.add)
                acc_spans.append(p0)
            # ACT-queue store for chunk 0, after every ACT scale op
            ooff0 = 0
            for ti, (it, F) in enumerate(planes):
                if ti == 0:
                    nc.scalar.dma_start(out=out[:, ooff0:ooff0 + F],
                                        in_=acc_spans[ti])
                else:
                    nc.sync.dma_start(out=out[:, ooff0:ooff0 + F],
                                      in_=acc_spans[ti])
                ooff0 += F

            # stats ride the tail of the SP ring (host only needs them
            # after the run, for the eigenvector sign check)
            nc.sync.dma_start(out=stats[:], in_=t1b[:])
    _split_sync_waits(nc)
    return nc


def _get_programs():
    global _programs
    if _programs is None:
        _programs = (_build_pass1(), _build_pass2())
    return _programs


def _get_fused():
    global _fused_program
    if _fused_program is None:
        _fused_program = _build_fused()
    return _fused_program


def _host_fold(stats8):
    """stats8: [B//SPC, 8, 130] f32 device-folded stats -> per-sample
    mu [B,4], sigma [B,4], comp [B,4] (reference-sign top eigenvector).

    Row 4s+m of a core's [8, 130] block holds M_s[4*(col//4)+m, col] for
    col<128 (exact copies of the block-diagonal entries) and colsum_s[m]
    at col 128.  Downstream matches the reference exactly: gram from
    (S - N mu mu^T) / (sigma sigma^T), comp = eigh(gram f32) top
    eigenvector on CPU jax.
    """
    st = stats8.astype(np.float64).reshape(B, 4, BST1)   # [b, m, col]
    t1 = st[:, :, :128].reshape(B, 4, 32, 4)             # [b, k, g, l]
    S = t1.sum(axis=2)                                   # [B, 4, 4]
    colsum = st[:, :, 128]                               # [B, 4]

    mu = colsum / NROWS
    e2 = np.einsum("bkk->bk", S) / NROWS
    var = np.maximum(e2 - mu * mu, 0.0)
    sigma = np.sqrt(var)
    denom = sigma[:, :, None] * sigma[:, None, :]
    gram = (S - NROWS * mu[:, :, None] * mu[:, None, :])
    with np.errstate(divide="ignore", invalid="ignore"):
        gram = np.where(denom > 0, gram / np.where(denom > 0, denom, 1.0), 0.0)

    # eigh with the same implementation/backend the reference uses (CPU jax)
    import jax
    import jax.numpy as jnp
    with jax.default_device(jax.devices("cpu")[0]):
        V = np.asarray(jnp.linalg.eigh(jnp.asarray(gram, jnp.float32))[1])
    comp = V[:, :, -1].astype(np.float64)                # top eigenvector
    return mu, sigma, comp


def _host_comp(stats8):
    return _host_fold(stats8)[2]


def _host_middle(stats8):
    """stats8 -> w [B, 4] f64, bias [B] f64 for the host-prescale path."""
    mu, sigma, comp = _host_fold(stats8)
    with np.errstate(divide="ignore", invalid="ignore"):
        w = np.where(sigma > 0, comp / np.where(sigma > 0, sigma, 1.0), 0.0)
    bias = -(mu * w).sum(axis=1)
    return w, bias


def _layouts(x16):
    """Build the gram-pass and plane-pass device layouts from fp16 x."""
    xp = np.zeros((B, 128, NBLK1, BST1), np.float16)
    xp[..., :128] = x16.reshape(B, NBLK1, 128, 128).transpose(0, 2, 1, 3)
    xp[..., 128] = 1.0
    xp = xp.reshape(B, 128, NBLK1 * BST1)

    xpl = x16.reshape(B, HO, 2, WO, 2, C // 4, 4).transpose(
        0, 1, 3, 6, 2, 4, 5)
    xpl = np.ascontiguousarray(xpl).reshape(B, 64, 49, 4, C)
    segs = []
    oo0 = 0
    for oo in FP2_TILES:
        seg = xpl[:, :, oo0:oo0 + oo].transpose(0, 1, 3, 2, 4)
        segs.append(seg.reshape(B, 64, 4 * oo * C))
        oo0 += oo
    x2h = np.concatenate(segs, axis=2)             # [B, 64, 49*4*C]
    return xp, x2h


def _layout_p2_scaled(x16, w, bias):
    """k-plane pass-2 layout with w_k folded into each plane and the bias
    folded into plane 0 (host knows w after pass 1; the scale rides the
    layout pass that exists anyway).  Returns x2h [B, 64, 49*4*C] fp16."""
    xpl = x16.reshape(B, HO, 2, WO, 2, C // 4, 4).transpose(
        0, 1, 3, 6, 2, 4, 5)
    xpl = np.ascontiguousarray(xpl).reshape(B, 3136, 4, C)
    xs = xpl.astype(np.float32)
    xs *= w.astype(np.float32)[:, None, :, None]
    xs[:, :, 0, :] += bias.astype(np.float32)[:, None, None]
    xpl = xs.astype(np.float16).reshape(B, 64, 49, 4, C)
    segs = []
    oo0 = 0
    for oo in P2_TILES:
        seg = xpl[:, :, oo0:oo0 + oo].transpose(0, 1, 3, 2, 4)
        segs.append(seg.reshape(B, 64, 4 * oo * C))
        oo0 += oo
    return np.concatenate(segs, axis=2)             # [B, 64, 49*4*C]


def _kernel_fused(x16):
    from concourse.bass_utils import run_bass_kernel_spmd

    ncf = _get_fused()
    core_ids = list(range(N_CORES))
    xp, x2h = _layouts(x16)
    cst = _host_consts()
    ins = []
    for c in range(N_CORES):
        pair = x2h[c * SPC:(c + 1) * SPC]
        ins.append({
            "xg": xp[c * SPC:(c + 1) * SPC],
            "xp": pair.reshape(128, 49 * 4 * C),
            "cst": cst,
        })
    kw = dict(trace=True, tmpdir=TRACE_DIRS.get("pass1")) if TRACE else {}
    r = run_bass_kernel_spmd(ncf, ins, core_ids, **kw)
    if TRACE:
        LAST_PROFILE["pass1_ns"] = r.exec_time_ns
        LAST_PROFILE["pass2_ns"] = 0

    # Sign fix: the device's power iteration returns comp * sign(comp.r8)
    # (even power of a positive-top-eig matrix applied to the fixed probe
    # r8), while the reference's eigh sign is whatever LAPACK produced.
    # comp_host from the device-folded stats tells us both.
    stats8 = np.stack([r.results[c]["stats"] for c in range(N_CORES)])
    comp = _host_comp(stats8)                     # [B, 4] reference-sign
    rp = np.array(R_PROBE, np.float64)
    flip = (comp @ rp) < 0                        # device sign != host sign

    outs = []
    for c in range(N_CORES):
        o = r.results[c]["out"].astype(np.float32).reshape(SPC, HO, WO, C)
        for s in range(SPC):
            if flip[c * SPC + s]:
                o[s] = -o[s]
        outs.append(o)
    return np.ascontiguousarray(np.concatenate(outs))


def kernel(x):
    from concourse.bass_utils import run_bass_kernel_spmd

    x = np.asarray(x)
    assert x.shape == (B, H, W, C), x.shape
    x16 = np.ascontiguousarray(x, dtype=np.float16)
    if FUSED:
        return _kernel_fused(x16)
    nc1, nc2 = _get_programs()
    core_ids = list(range(N_CORES))

    # pass-1 input: 128-row x 128-col blocks padded to 130 cols with a
    # ones column at 128 ([128 partitions, blocks]); row r = pix*2 + half
    xp = np.zeros((B, 128, NBLK1, BST1), np.float16)
    xp[..., :128] = x16.reshape(B, NBLK1, 128, 128).transpose(0, 2, 1, 3)
    xp[..., 128] = 1.0
    xp = xp.reshape(B, 128, NBLK1 * BST1)
    cst1 = _p1_consts()
    in1 = [{"x": xp[c * SPC:(c + 1) * SPC], "cst": cst1}
           for c in range(N_CORES)]
    kw1 = dict(trace=True, tmpdir=TRACE_DIRS.get("pass1")) if TRACE else {}
    r1 = run_bass_kernel_spmd(nc1, in1, core_ids, **kw1)
    if TRACE:
        LAST_PROFILE["pass1_ns"] = r1.exec_time_ns
    stats8 = np.stack([r1.results[c]["stats"] for c in range(N_CORES)])

    w, bias = _host_middle(stats8)
    x2h = _layout_p2_scaled(x16, w, bias)
    in2 = []
    for c in range(N_CORES):
        pair = x2h[c * SPC:(c + 1) * SPC]          # [2, 64, 49*4*C]
        in2.append({"x": pair.reshape(128, 49 * 4 * C)})
    kw2 = dict(trace=True, tmpdir=TRACE_DIRS.get("pass2")) if TRACE else {}
    r2 = run_bass_kernel_spmd(nc2, in2, core_ids, **kw2)
    if TRACE:
        LAST_PROFILE["pass2_ns"] = r2.exec_time_ns

    # gather: out[s*64+p64, oo*C+c'], outpix = p64*49+oo -> [B, HO, WO, C]
    outs = [r2.results[c]["out"].astype(np.float32).reshape(SPC, HO, WO, C)
            for c in range(N_CORES)]
    return np.ascontiguousarray(np.concatenate(outs))

